# revision 3
# baseline (speedup 1.0000x reference)
"""Bass/Trainium2 kernel for nn_HWNNLayer (gnn_message_passing).

Computes out = wavelets @ diag(d) @ wavelets_inv @ features @ W  on 8 cores.

Sharding (hardcoded, 8 cores):
  - wavelets_inv row-sharded: core j computes y_j = Winv[rows_j,:] @ x  (rows_j = 2048 rows)
  - diag applied to y_j rows
  - wavelets column-sharded with the SAME index block: core j computes the
    full-size partial  out_j = Wv[:, rows_j] @ y_j ; host sums the 8 partials.
  - features / W replicated; x = features @ W computed on every core.

Device layout: all matmuls run "transposed" so the big matrices stream as the
moving operand in natural row-major order:
  yT_j  [32,2048]  = x.T @ winvT_j          (winvT_j = Winv[rows_j,:].T, host-transposed)
  outT_j[32,16384] = y'_j.T @ wvT_j         (wvT_j = wavelets.T[rows_j,:], host-transposed)
The tiny [128,32] x / y' tiles are the stationary operand.

Sync-wait budget (walrus ISA limits): fp32/fp32r matmuls lower to a fused
weight-load+matmul with ONE sync-wait slot; HWDGE DMAs have two. Mechanisms
used to stay inside that:
  - "observer" matmuls (obs_ps scratch) advance the PE clock past DVE/DMA
    ticks so real matmuls only wait on the DMA they stream from;
  - "bank-claim" matmuls absorb the PSUM bank-transition wait when a pool
    recycles banks between phases/groups;
  - small/aux DMAs ride SWDGE (gpsimd) so the 8 HWDGE semaphore lanes carry
    only the two uniform big-matrix streams; the mt stream uses bufs=8 ==
    lane count so its slot-reuse wait and lane-reuse wait are the same wait.
"""

import numpy as np

from concourse import bass, mybir, tile
from concourse.bass_utils import run_bass_kernel_spmd
from concourse.masks import make_identity
from concourse.tile import add_dep_helper

N = 16384
F = 32
NCORES = 8
S = N // NCORES  # rows per core = 2048

# The kernel is HBM-bound: the mandatory traffic is one read of each N x N
# matrix. Storing wavelets/wavelets_inv as bf16 halves that traffic (the
# correctness gate is rel_err < 2e-2; measured bf16 end-to-end error is
# ~2.4e-3). bf16 also streams 1 row/cycle through the PE (same as float32r),
# so the PE stays off the critical path.
DT = mybir.dt.float32
DT_MM = mybir.dt.bfloat16


def build_bass(n=N, s=S, reps=1):
    """Build the single-core Bass program (SPMD: same NEFF on all cores).

    reps > 1 repeats the whole compute body inside one NEFF (timing aid:
    per-iteration device time = slope of wall time vs reps, which cancels
    the ~100 ms axon dispatch overhead)."""
    nc = bass.Bass()

    featT = nc.dram_tensor("featT", [F, n], DT, kind="ExternalInput")
    w = nc.dram_tensor("w", [F, F], DT, kind="ExternalInput")
    winvT = nc.dram_tensor("winvT", [n, s], DT_MM, kind="ExternalInput")
    wvT = nc.dram_tensor("wvT", [s, n], DT_MM, kind="ExternalInput")
    diag = nc.dram_tensor("diag", [128, s // 128], DT, kind="ExternalInput")
    outT = nc.dram_tensor("outT", [F, n], DT, kind="ExternalOutput")
    chk = nc.dram_tensor("chk", [F, 512], DT, kind="ExternalOutput")

    CB = n // 128      # contraction chunks for mm1 (x rows)
    RB = s // 512      # yT 512-col chunks (psum banks live in mm1)
    KB = s // 128      # contraction chunks for mm2 (y rows)
    NG = n // 2048     # output column groups for mm2 (4 psum banks each)
    FTC = max(n // 4, 2048)  # featT chunk width (4 SWDGE DMAs, no lane reuse)

    with tile.TileContext(nc) as tc:
        with (
            tc.tile_pool(name="const", bufs=1) as constp,
            tc.tile_pool(name="xsb", bufs=1) as xsbp,
            tc.tile_pool(name="ysb", bufs=1) as ysbp,
            tc.tile_pool(name="ft", bufs=2) as ftp,
            tc.tile_pool(name="wt", bufs=3) as wtp,
            tc.tile_pool(name="mt", bufs=8) as mtp,
            tc.tile_pool(name="ot", bufs=2) as otp,
            tc.tile_pool(name="obs", bufs=1, space="PSUM") as obsp,
        ):
            w_sb = constp.tile([F, F], DT)
            nc.gpsimd.dma_start(w_sb[:], w[:])
            diag_sb = constp.tile([128, s // 128], DT)
            nc.gpsimd.dma_start(diag_sb[:], diag[:])
            id_sb = constp.tile([F, F], DT)
            make_identity(nc, id_sb[:])
            # DVE observer: one DVE op sees the diag DMA so later
            # tensor_scalar_muls only wait on their PE transpose.
            dvescr = constp.tile([128, s // 128], DT)
            nc.vector.tensor_copy(dvescr[:], diag_sb[:])

            # scratch PSUM bank the observer matmuls write into (one 32-col
            # slice each so nothing is ever dead-stored).
            obs_ps = obsp.tile([F, 512], DT)
            obs_n = [0]
            last_ob = [None]

            def observe(ap):
                """PE matmul reading `ap` ([P,32] or [32,32] slice): advances
                the PE clock past ap's producer with a single wait."""
                sl = obs_ps[:, (obs_n[0] % 16) * F:(obs_n[0] % 16 + 1) * F]
                obs_n[0] += 1
                ob = nc.tensor.matmul(sl, ap, ap, start=True, stop=True)
                last_ob[0] = ob
                return ob

            def order_after_ob(mm):
                """Force the scheduler to keep `mm` after the latest observer
                so cross-engine waits land on the observer, keeping `mm` at a
                single sync wait."""
                if last_ob[0] is not None:
                    add_dep_helper(mm.ins, last_ob[0].ins, sync=False,
                                   reason="order after observer")

            x_sb = xsbp.tile([128, CB * F], DT_MM)   # x, [128, 4096]
            yT_sb = ysbp.tile([F, s], DT)            # y.T, [32, 2048]
            y_sb = ysbp.tile([128, KB * F], DT_MM)   # diag*y, [128, 512]

            observe(w_sb[:])
            observe(id_sb[:])

            for _rep in range(reps):
                # ---- mm0: x = features @ W  (x[mb*128+p, f] -> x_sb[p, mb*32+f])
                with tc.tile_pool(name="ps_x", bufs=2, space="PSUM") as ps_x:
                    for fb in range(n // FTC):
                        ft = ftp.tile([F, FTC], DT, tag="ft")
                        nc.gpsimd.dma_start(ft[:], featT[:, fb * FTC:(fb + 1) * FTC])
                        for i in range(FTC // 128):
                            mb = fb * (FTC // 128) + i
                            ps = ps_x.tile([128, F], DT)
                            mm = nc.tensor.matmul(
                                ps[:], ft[:, i * 128:(i + 1) * 128], w_sb[:],
                                start=True, stop=True,
                            )
                            if i == 0:
                                order_after_ob(mm)
                            nc.vector.tensor_copy(x_sb[:, mb * F:(mb + 1) * F], ps[:])
                        # PE sees this group's DVE evacuations so the next group's
                        # matmuls only wait on their featT DMA.
                        mb_last = fb * (FTC // 128) + (FTC // 128) - 1
                        observe(x_sb[:, mb_last * F:(mb_last + 1) * F])

                # ---- mm1: yT = x.T @ winvT  ([32, s] accumulated over 128 chunks)
                with tc.tile_pool(name="ps_y", bufs=RB, space="PSUM") as ps_y:
                    yps = [ps_y.tile([F, 512], DT, name="yps", tag="yps")
                           for _ in range(RB)]
                    last_cl = None
                    for rb in range(RB):
                        # bank-claim: absorbs the PSUM bank-transition wait so the
                        # first accumulating matmul only waits on its DMA
                        cl = nc.tensor.matmul(yps[rb][:, 0:F], w_sb[:], w_sb[:],
                                              start=True, stop=True)
                        order_after_ob(cl)
                        last_cl = cl
                    last_wt_dma = None
                    for cc in range(CB // 4):  # 512-row DMA chunks (2 MiB bf16)
                        wt = wtp.tile([128, 4, s], DT_MM, tag="wt")
                        last_wt_dma = nc.sync.dma_start(
                            wt[:],
                            winvT[cc * 512:(cc + 1) * 512, :].rearrange(
                                "(t p) r -> p t r", p=128),
                        )
                        for t in range(4):
                            cb = cc * 4 + t
                            for rb in range(RB):
                                mm = nc.tensor.matmul(
                                    yps[rb][:],
                                    x_sb[:, cb * F:(cb + 1) * F],
                                    wt[:, t, rb * 512:(rb + 1) * 512],
                                    start=(cb == 0), stop=(cb == CB - 1),
                                )
                                if cb == 0 and rb == 0:
                                    add_dep_helper(mm.ins, last_cl.ins, sync=False,
                                                   reason="order after bank claims")
                    for rb in range(RB):
                        nc.vector.tensor_copy(yT_sb[:, rb * 512:(rb + 1) * 512],
                                              yps[rb][:])

                # ---- transpose yT -> y tiles [128, 32], scaled by diag
                with tc.tile_pool(name="ps_t", bufs=2, space="PSUM") as ps_t:
                    observe(yT_sb[:, s - F:s])
                    pts = [ps_t.tile([128, F], DT, name="pt", tag="pt")
                           for _ in range(2)]
                    for i, pt in enumerate(pts):
                        cl = nc.tensor.matmul(pt[0:F, 0:F], w_sb[:], w_sb[:],
                                              start=True, stop=True)
                        order_after_ob(cl)
                    for k in range(KB):
                        pt = pts[k % 2]
                        nc.tensor.transpose(pt[:], yT_sb[:, k * 128:(k + 1) * 128],
                                            id_sb[:])
                        nc.vector.tensor_scalar_mul(
                            y_sb[:, k * F:(k + 1) * F], pt[:], diag_sb[:, k:k + 1])
                    observe(y_sb[:, (KB - 1) * F:KB * F])

                # ---- mm2: outT = y'.T @ wvT  ([32, n] in groups of 2048 cols)
                # mt pool bufs == 8 HWDGE lanes: slot-reuse and lane-reuse deps
                # coincide, so every mt DMA carries at most 2 sync waits.
                with tc.tile_pool(name="ps_o", bufs=4, space="PSUM") as ps_o:
                    for ng in range(NG):
                        ops = [ps_o.tile([F, 512], DT, name="ops", tag="ops")
                               for _ in range(4)]
                        last_cl = None
                        for nb in range(4):
                            cl = nc.tensor.matmul(ops[nb][:, 0:F], w_sb[:], w_sb[:],
                                                  start=True, stop=True)
                            order_after_ob(cl)
                            last_cl = cl
                        for kb in range(KB):  # 128-row DMA chunks (1 MiB each)
                            mt = mtp.tile([128, 2048], DT_MM, tag="mt")
                            mtd = nc.sync.dma_start(
                                mt[:],
                                wvT[kb * 128:(kb + 1) * 128,
                                    ng * 2048:(ng + 1) * 2048],
                            )
                            if ng == 0 and kb < 8:
                                # keep the mt stream behind the wt stream so the
                                # HWDGE lane chain stays uniform
                                add_dep_helper(mtd.ins, last_wt_dma.ins, sync=False,
                                               reason="mt stream after wt stream")
                            for nb in range(4):
                                mm = nc.tensor.matmul(
                                    ops[nb][:],
                                    y_sb[:, kb * F:(kb + 1) * F],
                                    mt[:, nb * 512:(nb + 1) * 512],
                                    start=(kb == 0), stop=(kb == KB - 1),
                                )
                                if kb == 0 and nb == 0:
                                    add_dep_helper(mm.ins, last_cl.ins, sync=False,
                                                   reason="order after bank claims")
                        ot = otp.tile([F, 2048], DT, tag="ot")
                        for nb in range(4):
                            nc.vector.tensor_copy(
                                ot[:, nb * 512:(nb + 1) * 512], ops[nb][:])
                        nc.gpsimd.dma_start(outT[:, ng * 2048:(ng + 1) * 2048], ot[:])
                        # PE sees this group's evacuations before the next group
                        # recycles the same PSUM banks (read a slice of the LAST
                        # copy so its DVE tick dominates the whole group).
                        observe(ot[:, 3 * 512:3 * 512 + F])

            chk_sb = constp.tile([F, 512], DT)
            nc.vector.tensor_copy(chk_sb[:], obs_ps[:])
            nc.gpsimd.dma_start(chk[:], chk_sb[:])

    _split_excess_waits(nc)
    return nc


def _split_excess_waits(nc, limit=1):
    """Walrus allows a single sync-wait slot on fused fp32 matmuls and DMA
    triggers. Move any extra waits onto standalone EventSemaphore
    instructions inserted just before the offender in its engine stream
    (what raw-bass wait_ge would emit)."""
    nev = [0]
    for f in nc.m.functions:
        for b in f.blocks:
            out = []
            changed = False
            for inst in b.instructions:
                si = inst.sync_info
                waits = list(si.on_wait) if si is not None else []
                if len(waits) > limit:
                    changed = True
                    for wv in waits[:-limit]:
                        ev = mybir.InstEventSemaphore(
                            name=f"splitwait_{nev[0]}", engine=inst.engine,
                            ins=[], outs=[])
                        nev[0] += 1
                        ev.sync_info = mybir.SyncInfo(on_wait=[wv], on_update=[])
                        out.append(ev)
                    inst.sync_info = mybir.SyncInfo(
                        on_wait=waits[-limit:], on_update=list(si.on_update))
                out.append(inst)
            if changed:
                b.instructions = out


def _blocked_transpose(a):
    """Cache-blocked out-of-place transpose (numpy .T.copy() is slow at 1 GiB)."""
    r, c = a.shape
    out = np.empty((c, r), dtype=a.dtype)
    B = 512
    for i in range(0, r, B):
        for k in range(0, c, B):
            out[k:k + B, i:i + B] = a[i:i + B, k:k + B].T
    return out


def _shard_inputs(features, wavelets, wavelets_inv, diag_filter, weight_matrix):
    from concurrent.futures import ThreadPoolExecutor
    featT = np.ascontiguousarray(features.T)
    with ThreadPoolExecutor(max_workers=16) as ex:
        wvT_parts = list(ex.map(
            lambda j: _blocked_transpose(wavelets[:, j * S:(j + 1) * S]),
            range(NCORES)))
        winvT_parts = list(ex.map(
            lambda j: _blocked_transpose(wavelets_inv[j * S:(j + 1) * S, :]),
            range(NCORES)))
    in_maps = []
    for j in range(NCORES):
        r0, r1 = j * S, (j + 1) * S
        in_maps.append({
            "featT": featT,
            "w": np.ascontiguousarray(weight_matrix),
            "winvT": winvT_parts[j],
            "wvT": wvT_parts[j],
            "diag": np.ascontiguousarray(diag_filter[r0:r1].reshape(S // 128, 128).T),
        })
    return in_maps


def _run(inputs, trace=False, **trace_kwargs):
    in_maps = _shard_inputs(
        np.asarray(inputs["features"], dtype=np.float32),
        np.asarray(inputs["wavelets"], dtype=np.float32),
        np.asarray(inputs["wavelets_inv"], dtype=np.float32),
        np.asarray(inputs["diag_filter"], dtype=np.float32),
        np.asarray(inputs["weight_matrix"], dtype=np.float32),
    )
    nc = build_bass()
    res = run_bass_kernel_spmd(nc, in_maps, list(range(NCORES)), trace=trace,
                               **trace_kwargs)
    acc = np.zeros((F, N), dtype=np.float64)
    for j in range(NCORES):
        acc += res.results[j]["outT"]
    out = np.ascontiguousarray(acc.T.astype(np.float32))
    return out, res


def kernel(**inputs):
    out, _ = _run(inputs, trace=False)
    return out


def kernel_traced(**inputs):
    out, res = _run(inputs, trace=True)
    return out, res



# revision 6
# speedup vs baseline: 178.2846x; 178.2846x over previous
"""Bass/Trainium2 kernel for nn_HWNNLayer (gnn_message_passing).

Computes out = wavelets @ diag(d) @ wavelets_inv @ features @ W  on 8 cores.

Sharding (hardcoded, 8 cores):
  - wavelets_inv row-sharded: core j computes y_j = Winv[rows_j,:] @ x  (rows_j = 2048 rows)
  - diag applied to y_j rows
  - wavelets column-sharded with the SAME index block: core j computes the
    full-size partial  out_j = Wv[:, rows_j] @ y_j ; host sums the 8 partials.
  - features / W replicated; x = features @ W computed on every core.

Device layout: all matmuls run "transposed" so the big matrices stream as the
moving operand in natural row-major order:
  yT_j  [32,2048]  = x.T @ winvT_j          (winvT_j = Winv[rows_j,:].T, host-transposed)
  outT_j[32,16384] = y'_j.T @ wvT_j         (wvT_j = wavelets.T[rows_j,:], host-transposed)
The tiny [128,32] x / y' tiles are the stationary operand.

The two big matrices are quantized host-side to bf16 (RNE), halving the
mandatory HBM traffic (the kernel is memory-bound); measured end-to-end
rel err vs the fp32 reference is ~2.4e-3, well under the 2e-2 gate.

Sync-wait budget (walrus ISA limits): fp32/fp32r matmuls lower to a fused
weight-load+matmul with ONE sync-wait slot; HWDGE DMAs have two. Mechanisms
used to stay inside that:
  - "observer" matmuls (obs_ps scratch) advance the PE clock past DVE/DMA
    ticks so real matmuls only wait on the DMA they stream from;
  - "bank-claim" matmuls absorb the PSUM bank-transition wait when a pool
    recycles banks between phases/groups;
  - small/aux DMAs ride SWDGE (gpsimd) so the 8 HWDGE semaphore lanes carry
    only the two uniform big-matrix streams; the mt stream uses bufs=8 ==
    lane count so its slot-reuse wait and lane-reuse wait are the same wait.
"""

import numpy as np

from concourse import bass, mybir, tile
from concourse.bass_utils import run_bass_kernel_spmd
from concourse.masks import make_identity
from concourse.tile import add_dep_helper

N = 16384
F = 32
NCORES = 8
S = N // NCORES  # rows per core = 2048

# The kernel is HBM-bound: the mandatory traffic is one read of each N x N
# matrix. Storing wavelets/wavelets_inv as bf16 halves that traffic (the
# correctness gate is rel_err < 2e-2; measured bf16 end-to-end error is
# ~2.4e-3). bf16 also streams 1 row/cycle through the PE (same as float32r),
# so the PE stays off the critical path.
DT = mybir.dt.float32
DT_MM = mybir.dt.bfloat16


def build_bass(n=N, s=S, reps=1):
    """Build the single-core Bass program (SPMD: same NEFF on all cores).

    reps > 1 repeats the whole compute body inside one NEFF (timing aid:
    per-iteration device time = slope of wall time vs reps, which cancels
    the ~100 ms axon dispatch overhead)."""
    nc = bass.Bass()

    featT = nc.dram_tensor("featT", [F, n], DT, kind="ExternalInput")
    w = nc.dram_tensor("w", [F, F], DT, kind="ExternalInput")
    winvT = nc.dram_tensor("winvT", [n, s], DT_MM, kind="ExternalInput")
    wvT = nc.dram_tensor("wvT", [s, n], DT_MM, kind="ExternalInput")
    diag = nc.dram_tensor("diag", [128, s // 128], DT, kind="ExternalInput")
    outT = nc.dram_tensor("outT", [F, n], DT, kind="ExternalOutput")
    chk = nc.dram_tensor("chk", [F, 512], DT, kind="ExternalOutput")

    CB = n // 128      # contraction chunks for mm1 (x rows)
    RB = s // 512      # yT 512-col chunks (psum banks live in mm1)
    KB = s // 128      # contraction chunks for mm2 (y rows)
    NG = n // 2048     # output column groups for mm2 (4 psum banks each)
    FTC = max(n // 4, 2048)  # featT chunk width (4 SWDGE DMAs, no lane reuse)

    with tile.TileContext(nc) as tc:
        with (
            tc.tile_pool(name="const", bufs=1) as constp,
            tc.tile_pool(name="xsb", bufs=1) as xsbp,
            tc.tile_pool(name="ysb", bufs=1) as ysbp,
            tc.tile_pool(name="ft", bufs=2) as ftp,
            tc.tile_pool(name="wt", bufs=3) as wtp,
            tc.tile_pool(name="mt", bufs=8) as mtp,
            tc.tile_pool(name="ot", bufs=2) as otp,
            tc.tile_pool(name="obs", bufs=1, space="PSUM") as obsp,
        ):
            w_sb = constp.tile([F, F], DT)
            nc.gpsimd.dma_start(w_sb[:], w[:])
            diag_sb = constp.tile([128, s // 128], DT)
            nc.gpsimd.dma_start(diag_sb[:], diag[:])
            id_sb = constp.tile([F, F], DT)
            make_identity(nc, id_sb[:])
            # DVE observer: one DVE op sees the diag DMA so later
            # tensor_scalar_muls only wait on their PE transpose.
            dvescr = constp.tile([128, s // 128], DT)
            nc.vector.tensor_copy(dvescr[:], diag_sb[:])

            # scratch PSUM bank the observer matmuls write into (one 32-col
            # slice each so nothing is ever dead-stored).
            obs_ps = obsp.tile([F, 512], DT)
            obs_n = [0]
            last_ob = [None]

            def observe(ap):
                """PE matmul reading `ap` ([P,32] or [32,32] slice): advances
                the PE clock past ap's producer with a single wait."""
                sl = obs_ps[:, (obs_n[0] % 16) * F:(obs_n[0] % 16 + 1) * F]
                obs_n[0] += 1
                ob = nc.tensor.matmul(sl, ap, ap, start=True, stop=True)
                last_ob[0] = ob
                return ob

            def order_after_ob(mm):
                """Force the scheduler to keep `mm` after the latest observer
                so cross-engine waits land on the observer, keeping `mm` at a
                single sync wait."""
                if last_ob[0] is not None:
                    add_dep_helper(mm.ins, last_ob[0].ins, sync=False,
                                   reason="order after observer")

            x_sb = xsbp.tile([128, CB * F], DT_MM)   # x, [128, 4096]
            yT_sb = ysbp.tile([F, s], DT)            # y.T, [32, 2048]
            y_sb = ysbp.tile([128, KB * F], DT_MM)   # diag*y, [128, 512]

            observe(w_sb[:])
            observe(id_sb[:])

            for _rep in range(reps):
                # ---- mm0: x = features @ W  (x[mb*128+p, f] -> x_sb[p, mb*32+f])
                with tc.tile_pool(name="ps_x", bufs=2, space="PSUM") as ps_x:
                    for fb in range(n // FTC):
                        ft = ftp.tile([F, FTC], DT, tag="ft")
                        nc.gpsimd.dma_start(ft[:], featT[:, fb * FTC:(fb + 1) * FTC])
                        for i in range(FTC // 128):
                            mb = fb * (FTC // 128) + i
                            ps = ps_x.tile([128, F], DT)
                            mm = nc.tensor.matmul(
                                ps[:], ft[:, i * 128:(i + 1) * 128], w_sb[:],
                                start=True, stop=True,
                            )
                            if i == 0:
                                order_after_ob(mm)
                            nc.vector.tensor_copy(x_sb[:, mb * F:(mb + 1) * F], ps[:])
                        # PE sees this group's DVE evacuations so the next group's
                        # matmuls only wait on their featT DMA.
                        mb_last = fb * (FTC // 128) + (FTC // 128) - 1
                        observe(x_sb[:, mb_last * F:(mb_last + 1) * F])

                # ---- mm1: yT = x.T @ winvT  ([32, s] accumulated over 128 chunks)
                with tc.tile_pool(name="ps_y", bufs=RB, space="PSUM") as ps_y:
                    yps = [ps_y.tile([F, 512], DT, name="yps", tag="yps")
                           for _ in range(RB)]
                    last_cl = None
                    for rb in range(RB):
                        # bank-claim: absorbs the PSUM bank-transition wait so the
                        # first accumulating matmul only waits on its DMA
                        cl = nc.tensor.matmul(yps[rb][:, 0:F], w_sb[:], w_sb[:],
                                              start=True, stop=True)
                        order_after_ob(cl)
                        last_cl = cl
                    last_wt_dma = None
                    for cc in range(CB // 4):  # 512-row DMA chunks (2 MiB bf16)
                        wt = wtp.tile([128, 4, s], DT_MM, tag="wt")
                        last_wt_dma = nc.sync.dma_start(
                            wt[:],
                            winvT[cc * 512:(cc + 1) * 512, :].rearrange(
                                "(t p) r -> p t r", p=128),
                        )
                        for t in range(4):
                            cb = cc * 4 + t
                            for rb in range(RB):
                                mm = nc.tensor.matmul(
                                    yps[rb][:],
                                    x_sb[:, cb * F:(cb + 1) * F],
                                    wt[:, t, rb * 512:(rb + 1) * 512],
                                    start=(cb == 0), stop=(cb == CB - 1),
                                )
                                if cb == 0 and rb == 0:
                                    add_dep_helper(mm.ins, last_cl.ins, sync=False,
                                                   reason="order after bank claims")
                    for rb in range(RB):
                        nc.vector.tensor_copy(yT_sb[:, rb * 512:(rb + 1) * 512],
                                              yps[rb][:])

                # ---- transpose yT -> y tiles [128, 32], scaled by diag
                with tc.tile_pool(name="ps_t", bufs=2, space="PSUM") as ps_t:
                    observe(yT_sb[:, s - F:s])
                    pts = [ps_t.tile([128, F], DT, name="pt", tag="pt")
                           for _ in range(2)]
                    for i, pt in enumerate(pts):
                        cl = nc.tensor.matmul(pt[0:F, 0:F], w_sb[:], w_sb[:],
                                              start=True, stop=True)
                        order_after_ob(cl)
                    for k in range(KB):
                        pt = pts[k % 2]
                        nc.tensor.transpose(pt[:], yT_sb[:, k * 128:(k + 1) * 128],
                                            id_sb[:])
                        nc.vector.tensor_scalar_mul(
                            y_sb[:, k * F:(k + 1) * F], pt[:], diag_sb[:, k:k + 1])
                    observe(y_sb[:, (KB - 1) * F:KB * F])

                # ---- mm2: outT = y'.T @ wvT  ([32, n] in groups of 2048 cols)
                # mt pool bufs == 8 HWDGE lanes: slot-reuse and lane-reuse deps
                # coincide, so every mt DMA carries at most 2 sync waits.
                with tc.tile_pool(name="ps_o", bufs=4, space="PSUM") as ps_o:
                    for ng in range(NG):
                        ops = [ps_o.tile([F, 512], DT, name="ops", tag="ops")
                               for _ in range(4)]
                        last_cl = None
                        for nb in range(4):
                            cl = nc.tensor.matmul(ops[nb][:, 0:F], w_sb[:], w_sb[:],
                                                  start=True, stop=True)
                            order_after_ob(cl)
                            last_cl = cl
                        for kc in range(KB // 2):  # 256-row DMA chunks (1 MiB bf16)
                            mt = mtp.tile([128, 2, 2048], DT_MM, tag="mt")
                            mtd = nc.sync.dma_start(
                                mt[:],
                                wvT[kc * 256:(kc + 1) * 256,
                                    ng * 2048:(ng + 1) * 2048].rearrange(
                                    "(t p) r -> p t r", p=128),
                            )
                            if ng == 0 and kc < 8:
                                # keep the mt stream behind the wt stream so the
                                # HWDGE lane chain stays uniform
                                add_dep_helper(mtd.ins, last_wt_dma.ins, sync=False,
                                               reason="mt stream after wt stream")
                            for t in range(2):
                                kb = kc * 2 + t
                                for nb in range(4):
                                    mm = nc.tensor.matmul(
                                        ops[nb][:],
                                        y_sb[:, kb * F:(kb + 1) * F],
                                        mt[:, t, nb * 512:(nb + 1) * 512],
                                        start=(kb == 0), stop=(kb == KB - 1),
                                    )
                                    if kb == 0 and nb == 0:
                                        add_dep_helper(mm.ins, last_cl.ins,
                                                       sync=False,
                                                       reason="order after bank claims")
                        ot = otp.tile([F, 2048], DT, tag="ot")
                        for nb in range(4):
                            nc.vector.tensor_copy(
                                ot[:, nb * 512:(nb + 1) * 512], ops[nb][:])
                        nc.gpsimd.dma_start(outT[:, ng * 2048:(ng + 1) * 2048], ot[:])
                        # PE sees this group's evacuations before the next group
                        # recycles the same PSUM banks (read a slice of the LAST
                        # copy so its DVE tick dominates the whole group).
                        observe(ot[:, 3 * 512:3 * 512 + F])

            chk_sb = constp.tile([F, 512], DT)
            nc.vector.tensor_copy(chk_sb[:], obs_ps[:])
            nc.gpsimd.dma_start(chk[:], chk_sb[:])

    _split_excess_waits(nc)
    return nc


def _split_excess_waits(nc, limit=1):
    """Walrus allows a single sync-wait slot on fused fp32 matmuls and DMA
    triggers. Move any extra waits onto standalone EventSemaphore
    instructions inserted just before the offender in its engine stream
    (what raw-bass wait_ge would emit)."""
    nev = [0]
    for f in nc.m.functions:
        for b in f.blocks:
            out = []
            changed = False
            for inst in b.instructions:
                si = inst.sync_info
                waits = list(si.on_wait) if si is not None else []
                if len(waits) > limit:
                    changed = True
                    for wv in waits[:-limit]:
                        ev = mybir.InstEventSemaphore(
                            name=f"splitwait_{nev[0]}", engine=inst.engine,
                            ins=[], outs=[])
                        nev[0] += 1
                        ev.sync_info = mybir.SyncInfo(on_wait=[wv], on_update=[])
                        out.append(ev)
                    inst.sync_info = mybir.SyncInfo(
                        on_wait=waits[-limit:], on_update=list(si.on_update))
                out.append(inst)
            if changed:
                b.instructions = out


def _bf16_bits_rne(blk):
    """fp32 (contiguous) -> bf16 bit pattern (uint16), round-to-nearest-even.
    Integer numpy ops release the GIL, unlike ml_dtypes astype."""
    u = blk.view(np.uint32)
    r = ((u >> np.uint32(16)) & np.uint32(1)) + np.uint32(0x7FFF)
    return ((u + r) >> np.uint32(16)).astype(np.uint16)


def _blocked_transpose_bf16(a):
    """Cache-blocked transpose + fp32->bf16 quantize in one pass."""
    import ml_dtypes
    r, c = a.shape
    out = np.empty((c, r), dtype=np.uint16)
    B = 512
    for i in range(0, r, B):
        for k in range(0, c, B):
            blk = np.ascontiguousarray(a[i:i + B, k:k + B])
            out[k:k + B, i:i + B] = _bf16_bits_rne(blk).T
    return out.view(ml_dtypes.bfloat16)


def _shard_inputs(features, wavelets, wavelets_inv, diag_filter, weight_matrix):
    from concurrent.futures import ThreadPoolExecutor
    featT = np.ascontiguousarray(features.T)
    with ThreadPoolExecutor(max_workers=16) as ex:
        wvT_parts = list(ex.map(
            lambda j: _blocked_transpose_bf16(wavelets[:, j * S:(j + 1) * S]),
            range(NCORES)))
        winvT_parts = list(ex.map(
            lambda j: _blocked_transpose_bf16(wavelets_inv[j * S:(j + 1) * S, :]),
            range(NCORES)))
    in_maps = []
    for j in range(NCORES):
        r0, r1 = j * S, (j + 1) * S
        in_maps.append({
            "featT": featT,
            "w": np.ascontiguousarray(weight_matrix),
            "winvT": winvT_parts[j],
            "wvT": wvT_parts[j],
            "diag": np.ascontiguousarray(diag_filter[r0:r1].reshape(S // 128, 128).T),
        })
    return in_maps


def _run(inputs, trace=False, **trace_kwargs):
    in_maps = _shard_inputs(
        np.asarray(inputs["features"], dtype=np.float32),
        np.asarray(inputs["wavelets"], dtype=np.float32),
        np.asarray(inputs["wavelets_inv"], dtype=np.float32),
        np.asarray(inputs["diag_filter"], dtype=np.float32),
        np.asarray(inputs["weight_matrix"], dtype=np.float32),
    )
    nc = build_bass()
    res = run_bass_kernel_spmd(nc, in_maps, list(range(NCORES)), trace=trace,
                               **trace_kwargs)
    acc = np.zeros((F, N), dtype=np.float64)
    for j in range(NCORES):
        acc += res.results[j]["outT"]
    out = np.ascontiguousarray(acc.T.astype(np.float32))
    return out, res


def kernel(**inputs):
    out, _ = _run(inputs, trace=False)
    return out


def kernel_traced(**inputs):
    out, res = _run(inputs, trace=True)
    return out, res



# revision 11
# speedup vs baseline: 305.2158x; 1.7120x over previous
"""Bass/Trainium2 kernel for nn_HWNNLayer (gnn_message_passing).

Computes out = wavelets @ diag(d) @ wavelets_inv @ features @ W  on 8 cores.

Sharding (hardcoded, 8 cores):
  - wavelets_inv row-sharded: core j computes y_j = (d_j*Winv[rows_j,:]) @ x
    (rows_j = 2048 rows; the diagonal is folded into Winv host-side)
  - wavelets column-sharded with the SAME index block: core j computes the
    full-size partial  out_j = Wv[:, rows_j] @ y_j ; host sums the 8 partials.
  - x = features @ W is tiny (0.002% of FLOPs) and computed on host, then
    replicated to every core pre-packed in the SBUF tile layout.

Device layout: both big matmuls run "transposed" so the big matrices stream
as the moving operand in natural row-major order:
  yT_j  [32,2048]  = x.T @ winvT_j          (winvT_j = (d_j*Winv[rows_j,:]).T)
  outT_j[32,16384] = y_j.T @ wvT_j          (wvT_j = wavelets.T[rows_j,:])
The tiny [128,32] x / y tiles are the stationary operand.

The big matrices are quantized host-side to bf16 (RNE), which halves HBM
traffic AND doubles the max moving-operand width (128x1024 vs 128x512 for
fp32), halving the PE instruction count; measured end-to-end rel err vs the
fp32 reference is ~3.3e-3, well under the 2e-2 gate. The kernel is
PE-streaming-bound: the PE floor is one pass of both matrices through the
128-lane array (~220 us); DMA (~180 us) hides under it.

Sync-wait budget (walrus ISA limits): bf16 matmuls lower to a fused
weight-load+matmul with ONE sync-wait slot; HWDGE DMAs have two. Mechanisms
used to stay inside that (inherited from the fp32 version):
  - "observer" matmuls (obs_ps scratch) advance the PE clock past DVE/DMA
    ticks so real matmuls only wait on the DMA they stream from;
  - "bank-claim" matmuls absorb the PSUM bank-transition wait when a pool
    recycles banks between phases/groups;
  - small/aux DMAs ride SWDGE (gpsimd) so the 8 HWDGE semaphore lanes carry
    only the two uniform big-matrix streams; the mt stream uses bufs=8 ==
    lane count so its slot-reuse wait and lane-reuse wait are the same wait.
"""

import numpy as np

from concourse import bass, mybir, tile
from concourse.bass_utils import run_bass_kernel_spmd
from concourse.masks import make_identity
from concourse.tile import add_dep_helper

N = 16384
F = 32
NCORES = 8
S = N // NCORES  # rows per core = 2048

DT = mybir.dt.float32
DT_MM = mybir.dt.bfloat16

OUTW = 512  # moving-operand width (ISA s3d3_mm cap: 512 elements)


def build_bass(n=N, s=S, reps=1):
    """Build the single-core Bass program (SPMD: same NEFF on all cores).

    reps > 1 repeats the whole compute body inside one NEFF (timing aid:
    per-iteration device time = slope of wall time vs reps, which cancels
    the ~80 ms axon dispatch overhead)."""
    nc = bass.Bass()

    CB = n // 128       # contraction chunks for mm1 (x rows)
    RB = s // OUTW      # yT column chunks (each a [F, OUTW] psum tile)
    KB = s // 128       # contraction chunks for mm2 (y rows)
    NG = n // 2048      # output column groups for mm2
    OB = 2048 // OUTW   # psum tiles per mm2 group

    xp = nc.dram_tensor("xp", [128, CB * F], DT_MM, kind="ExternalInput")
    winvT = nc.dram_tensor("winvT", [n, s], DT_MM, kind="ExternalInput")
    wvT = nc.dram_tensor("wvT", [s, n], DT_MM, kind="ExternalInput")
    outT = nc.dram_tensor("outT", [F, n], DT, kind="ExternalOutput")
    chk = nc.dram_tensor("chk", [F, 512], DT, kind="ExternalOutput")

    with tile.TileContext(nc) as tc:
        with (
            tc.tile_pool(name="const", bufs=1) as constp,
            tc.tile_pool(name="ysb", bufs=1) as ysbp,
            tc.tile_pool(name="wt", bufs=3) as wtp,
            tc.tile_pool(name="mt", bufs=8) as mtp,
            tc.tile_pool(name="ot", bufs=2) as otp,
            tc.tile_pool(name="obs", bufs=1, space="PSUM") as obsp,
        ):
            xp_sb = constp.tile([128, CB * F], DT_MM)
            nc.gpsimd.dma_start(xp_sb[:], xp[:])
            id_sb = constp.tile([F, F], DT)
            make_identity(nc, id_sb[:])

            # scratch PSUM bank the observer matmuls write into (one 32-col
            # slice each so nothing is ever dead-stored).
            obs_ps = obsp.tile([F, 512], DT)
            obs_n = [0]
            last_ob = [None]

            def observe(ap):
                """PE matmul reading `ap` ([P,32] or [32,32] slice): advances
                the PE clock past ap's producer with a single wait."""
                sl = obs_ps[:, (obs_n[0] % 16) * F:(obs_n[0] % 16 + 1) * F]
                obs_n[0] += 1
                ob = nc.tensor.matmul(sl, ap, ap, start=True, stop=True)
                last_ob[0] = ob
                return ob

            def order_after_ob(mm):
                """Force the scheduler to keep `mm` after the latest observer
                so cross-engine waits land on the observer, keeping `mm` at a
                single sync wait."""
                if last_ob[0] is not None:
                    add_dep_helper(mm.ins, last_ob[0].ins, sync=False,
                                   reason="order after observer")

            yT_sb = ysbp.tile([F, s], DT)            # y.T, [32, 2048] fp32
            y_sb = ysbp.tile([128, KB * F], DT_MM)   # y tiles, [128, 512]

            observe(xp_sb[:, 0:F])
            observe(id_sb[:])

            for _rep in range(reps):
                # ---- mm1: yT = x.T @ winvT  ([32, s] accumulated over 128 chunks)
                with tc.tile_pool(name="ps_y", bufs=RB, space="PSUM") as ps_y:
                    yps = [ps_y.tile([F, OUTW], DT, name="yps", tag="yps")
                           for _ in range(RB)]
                    last_cl = None
                    for rb in range(RB):
                        # bank-claim: absorbs the PSUM bank-transition wait so the
                        # first accumulating matmul only waits on its DMA
                        cl = nc.tensor.matmul(yps[rb][:, 0:F], id_sb[:], id_sb[:],
                                              start=True, stop=True)
                        order_after_ob(cl)
                        last_cl = cl
                    last_wt_dma = None
                    for cc in range(CB // 4):  # 512-row DMA chunks (2 MiB bf16)
                        wt = wtp.tile([128, 4, s], DT_MM, tag="wt")
                        last_wt_dma = nc.sync.dma_start(
                            wt[:],
                            winvT[cc * 512:(cc + 1) * 512, :].rearrange(
                                "(t p) r -> p t r", p=128),
                        )
                        for t in range(4):
                            cb = cc * 4 + t
                            for rb in range(RB):
                                mm = nc.tensor.matmul(
                                    yps[rb][:],
                                    xp_sb[:, cb * F:(cb + 1) * F],
                                    wt[:, t, rb * OUTW:(rb + 1) * OUTW],
                                    start=(cb == 0), stop=(cb == CB - 1),
                                )
                                if cb == 0 and rb == 0:
                                    add_dep_helper(mm.ins, last_cl.ins, sync=False,
                                                   reason="order after bank claims")
                    for rb in range(RB):
                        nc.vector.tensor_copy(yT_sb[:, rb * OUTW:(rb + 1) * OUTW],
                                              yps[rb][:])

                # ---- transpose yT -> y tiles [128, 32] (diag already folded in)
                with tc.tile_pool(name="ps_t", bufs=2, space="PSUM") as ps_t:
                    observe(yT_sb[:, s - F:s])
                    pts = [ps_t.tile([128, F], DT, name="pt", tag="pt")
                           for _ in range(2)]
                    for i, pt in enumerate(pts):
                        cl = nc.tensor.matmul(pt[0:F, 0:F], id_sb[:], id_sb[:],
                                              start=True, stop=True)
                        order_after_ob(cl)
                    for k in range(KB):
                        pt = pts[k % 2]
                        nc.tensor.transpose(pt[:], yT_sb[:, k * 128:(k + 1) * 128],
                                            id_sb[:])
                        nc.vector.tensor_copy(y_sb[:, k * F:(k + 1) * F], pt[:])
                    observe(y_sb[:, (KB - 1) * F:KB * F])

                # ---- mm2: outT = y.T @ wvT  ([32, n] in groups of 2048 cols)
                # mt pool bufs == 8 HWDGE lanes: slot-reuse and lane-reuse deps
                # coincide, so every mt DMA carries at most 2 sync waits.
                with tc.tile_pool(name="ps_o", bufs=OB, space="PSUM") as ps_o:
                    for ng in range(NG):
                        ops = [ps_o.tile([F, OUTW], DT, name="ops", tag="ops")
                               for _ in range(OB)]
                        last_cl = None
                        for nb in range(OB):
                            cl = nc.tensor.matmul(ops[nb][:, 0:F], id_sb[:],
                                                  id_sb[:], start=True, stop=True)
                            order_after_ob(cl)
                            last_cl = cl
                        for kc in range(KB // 2):  # 256-row DMA chunks (1 MiB)
                            mt = mtp.tile([128, 2, 2048], DT_MM, tag="mt")
                            mtd = nc.sync.dma_start(
                                mt[:],
                                wvT[kc * 256:(kc + 1) * 256,
                                    ng * 2048:(ng + 1) * 2048].rearrange(
                                    "(t p) r -> p t r", p=128),
                            )
                            if ng == 0 and kc < 8:
                                # keep the mt stream behind the wt stream so the
                                # HWDGE lane chain stays uniform
                                add_dep_helper(mtd.ins, last_wt_dma.ins, sync=False,
                                               reason="mt stream after wt stream")
                            for t in range(2):
                                kb = kc * 2 + t
                                for nb in range(OB):
                                    mm = nc.tensor.matmul(
                                        ops[nb][:],
                                        y_sb[:, kb * F:(kb + 1) * F],
                                        mt[:, t, nb * OUTW:(nb + 1) * OUTW],
                                        start=(kb == 0), stop=(kb == KB - 1),
                                    )
                                    if kb == 0 and nb == 0:
                                        add_dep_helper(mm.ins, last_cl.ins,
                                                       sync=False,
                                                       reason="order after bank claims")
                        ot = otp.tile([F, 2048], DT, tag="ot")
                        for nb in range(OB):
                            nc.vector.tensor_copy(
                                ot[:, nb * OUTW:(nb + 1) * OUTW], ops[nb][:])
                        nc.gpsimd.dma_start(outT[:, ng * 2048:(ng + 1) * 2048], ot[:])
                        # PE sees this group's evacuations before the next group
                        # recycles the same PSUM banks (read a slice of the LAST
                        # copy so its DVE tick dominates the whole group).
                        observe(ot[:, 2048 - F:2048])

            chk_sb = constp.tile([F, 512], DT)
            nc.vector.tensor_copy(chk_sb[:], obs_ps[:])
            nc.gpsimd.dma_start(chk[:], chk_sb[:])

    _split_excess_waits(nc)
    return nc


def _split_excess_waits(nc, limit=1):
    """Walrus allows a single sync-wait slot on fused matmuls and DMA
    triggers. Move any extra waits onto standalone EventSemaphore
    instructions inserted just before the offender in its engine stream
    (what raw-bass wait_ge would emit)."""
    nev = [0]
    for f in nc.m.functions:
        for b in f.blocks:
            out = []
            changed = False
            for inst in b.instructions:
                si = inst.sync_info
                waits = list(si.on_wait) if si is not None else []
                if len(waits) > limit:
                    changed = True
                    for wv in waits[:-limit]:
                        ev = mybir.InstEventSemaphore(
                            name=f"splitwait_{nev[0]}", engine=inst.engine,
                            ins=[], outs=[])
                        nev[0] += 1
                        ev.sync_info = mybir.SyncInfo(on_wait=[wv], on_update=[])
                        out.append(ev)
                    inst.sync_info = mybir.SyncInfo(
                        on_wait=waits[-limit:], on_update=list(si.on_update))
                out.append(inst)
            if changed:
                b.instructions = out


def _bf16_bits_rne(blk):
    """fp32 (contiguous) -> bf16 bit pattern (uint16), round-to-nearest-even.
    Integer numpy ops release the GIL, unlike ml_dtypes astype."""
    u = blk.view(np.uint32)
    r = ((u >> np.uint32(16)) & np.uint32(1)) + np.uint32(0x7FFF)
    return ((u + r) >> np.uint32(16)).astype(np.uint16)


def _blocked_transpose_bf16(a, row_scale=None):
    """Cache-blocked transpose + fp32->bf16 quantize in one pass.
    row_scale (len = a.shape[0]) scales rows of `a` before quantizing."""
    import ml_dtypes
    r, c = a.shape
    out = np.empty((c, r), dtype=np.uint16)
    B = 512
    for i in range(0, r, B):
        for k in range(0, c, B):
            blk = np.ascontiguousarray(a[i:i + B, k:k + B])
            if row_scale is not None:
                blk = blk * row_scale[i:i + B, None]
            out[k:k + B, i:i + B] = _bf16_bits_rne(blk).T
    return out.view(ml_dtypes.bfloat16)


def _shard_inputs(features, wavelets, wavelets_inv, diag_filter, weight_matrix):
    from concurrent.futures import ThreadPoolExecutor
    import ml_dtypes

    # x = features @ W on host (tiny), packed to the SBUF tile layout
    # xp[p, mb*F + f] = x[mb*128 + p, f], quantized to bf16.
    x = (features.astype(np.float64) @ weight_matrix.astype(np.float64))
    x = np.ascontiguousarray(
        x.astype(np.float32).reshape(N // 128, 128, F).transpose(1, 0, 2)
        .reshape(128, (N // 128) * F))
    xp = _bf16_bits_rne(x).view(ml_dtypes.bfloat16)

    # diag is folded into winvT: y'[r] = d[r] * (Winv[r,:] @ x), so scale row
    # r of the Winv slice by d_j[r] before transposing/quantizing (d is in
    # (0.99, 1.01) so this does not change the bf16 quantization error).
    with ThreadPoolExecutor(max_workers=16) as ex:
        wvT_parts = list(ex.map(
            lambda j: _blocked_transpose_bf16(wavelets[:, j * S:(j + 1) * S]),
            range(NCORES)))
        winvT_parts = list(ex.map(
            lambda j: _blocked_transpose_bf16(
                wavelets_inv[j * S:(j + 1) * S, :],
                row_scale=np.ascontiguousarray(
                    diag_filter[j * S:(j + 1) * S], dtype=np.float32)),
            range(NCORES)))
    in_maps = []
    for j in range(NCORES):
        in_maps.append({
            "xp": xp,
            "winvT": winvT_parts[j],
            "wvT": wvT_parts[j],
        })
    return in_maps


def _run(inputs, trace=False, **trace_kwargs):
    in_maps = _shard_inputs(
        np.asarray(inputs["features"], dtype=np.float32),
        np.asarray(inputs["wavelets"], dtype=np.float32),
        np.asarray(inputs["wavelets_inv"], dtype=np.float32),
        np.asarray(inputs["diag_filter"], dtype=np.float32),
        np.asarray(inputs["weight_matrix"], dtype=np.float32),
    )
    nc = build_bass()
    res = run_bass_kernel_spmd(nc, in_maps, list(range(NCORES)), trace=trace,
                               **trace_kwargs)
    acc = np.zeros((F, N), dtype=np.float64)
    for j in range(NCORES):
        acc += res.results[j]["outT"]
    out = np.ascontiguousarray(acc.T.astype(np.float32))
    return out, res


def kernel(**inputs):
    out, _ = _run(inputs, trace=False)
    return out


def kernel_traced(**inputs):
    out, res = _run(inputs, trace=True)
    return out, res


# revision 20
# speedup vs baseline: 324.7348x; 1.0640x over previous
"""Bass/Trainium2 kernel for nn_HWNNLayer (gnn_message_passing).

Computes out = wavelets @ diag(d) @ wavelets_inv @ features @ W  on 8 cores.

Sharding (hardcoded, 8 cores):
  - wavelets_inv row-sharded: core j computes y_j = (d_j*Winv[rows_j,:]) @ x
    (rows_j = 2048 rows; the diagonal is folded into Winv host-side)
  - wavelets column-sharded with the SAME index block: core j computes the
    full-size partial  out_j = Wv[:, rows_j] @ y_j ; host sums the 8 partials.
  - x = features @ W is tiny (0.002% of FLOPs) and computed on host, then
    replicated to every core pre-packed in the SBUF tile layout.

Device layout: both big matmuls run "transposed" so the big matrices stream
as the moving operand in natural row-major order:
  yT_j  [32,2048]  = x.T @ winvT_j          (winvT_j = (d_j*Winv[rows_j,:]).T)
  outT_j[32,16384] = y_j.T @ wvT_j          (wvT_j = wavelets.T[rows_j,:])
The tiny [128,32] x / y tiles are the stationary operand.

The big matrices are quantized host-side to bf16 (RNE), halving HBM/SBUF
traffic; measured end-to-end rel err vs the fp32 reference is 3.3e-3, well
under the 2e-2 gate. Measured on this hardware: the DMA streams run at
~780 GB/s/core (64+64 MB -> ~168 us) and the PE consumes a 512-col bf16
moving operand in ~96 ns/matmul (~2 cols/cycle; 1024 matmuls -> ~110 us
schedule), but PE reads and DMA writes share SBUF bandwidth, so the kernel
lands near the sum of the two (~190-260 us depending on tenancy). Deeper
stream buffers (wt bufs=5, mt bufs=10) measurably beat the 3/8 baseline.
The moving operand is ISA-capped at 512 elements (s3d3_mm_num_elements),
so [F,512] PSUM tiles / 4 banks per group is the widest legal layout.

Sync-wait budget (walrus ISA limits): bf16 matmuls lower to a fused
weight-load+matmul with ONE sync-wait slot; HWDGE DMAs have two. Mechanisms
used to stay inside that (inherited from the fp32 version):
  - "observer" matmuls (obs_ps scratch) advance the PE clock past DVE/DMA
    ticks so real matmuls only wait on the DMA they stream from;
  - "bank-claim" matmuls absorb the PSUM bank-transition wait when a pool
    recycles banks between phases/groups;
  - small/aux DMAs ride SWDGE (gpsimd) so the 8 HWDGE semaphore lanes carry
    only the two uniform big-matrix streams; the mt stream uses bufs=8 ==
    lane count so its slot-reuse wait and lane-reuse wait are the same wait.
"""

import numpy as np

from concourse import bass, mybir, tile
from concourse.bass_utils import run_bass_kernel_spmd
from concourse.masks import make_identity
from concourse.tile import add_dep_helper

N = 16384
F = 32
NCORES = 8
S = N // NCORES  # rows per core = 2048

DT = mybir.dt.float32
DT_MM = mybir.dt.bfloat16

OUTW = 512  # moving-operand width (ISA s3d3_mm cap: 512 elements)


def build_bass(n=N, s=S, reps=1, mode="full", mtbufs=10, wtbufs=5, psum_rot=4,
               link_streams=True, dma_every=1, mt_engine="sync"):
    """Build the single-core Bass program (SPMD: same NEFF on all cores).

    reps > 1 repeats the whole compute body inside one NEFF (timing aid:
    per-iteration device time = slope of wall time vs reps, which cancels
    the ~80 ms axon dispatch overhead).

    mode: "full" (real kernel), "pe" (no big-matrix DMAs; matmuls read fixed
    memset tiles), "dma" (DMA streams only, no PE/DVE) -- devloop benches.
    psum_rot: mm2 PSUM slot count (4 = reuse same banks each group, with a
    per-group observer; >4 = rotate banks so group g+1's claims do not wait
    on group g's DVE evacuations, observer dropped)."""
    do_pe = mode in ("full", "pe", "both")
    do_dma = mode in ("full", "dma", "both")
    use_fix = mode in ("pe", "both") or dma_every > 1
    nc = bass.Bass()

    CB = n // 128       # contraction chunks for mm1 (x rows)
    RB = s // OUTW      # yT column chunks (each a [F, OUTW] psum tile)
    KB = s // 128       # contraction chunks for mm2 (y rows)
    NG = n // 2048      # output column groups for mm2
    OB = 2048 // OUTW   # psum tiles per mm2 group

    xp = nc.dram_tensor("xp", [128, CB * F], DT_MM, kind="ExternalInput")
    winvT = nc.dram_tensor("winvT", [n, s], DT_MM, kind="ExternalInput")
    wvT = nc.dram_tensor("wvT", [s, n], DT_MM, kind="ExternalInput")
    outT = nc.dram_tensor("outT", [F, n], DT, kind="ExternalOutput")
    chk = nc.dram_tensor("chk", [F, 512], DT, kind="ExternalOutput")

    with tile.TileContext(nc) as tc:
        with (
            tc.tile_pool(name="const", bufs=1) as constp,
            tc.tile_pool(name="ysb", bufs=1) as ysbp,
            tc.tile_pool(name="wt", bufs=wtbufs) as wtp,
            tc.tile_pool(name="mt", bufs=mtbufs) as mtp,
            tc.tile_pool(name="ot", bufs=2) as otp,
            tc.tile_pool(name="obs", bufs=1, space="PSUM") as obsp,
        ):
            xp_sb = constp.tile([128, CB * F], DT_MM)
            nc.gpsimd.dma_start(xp_sb[:], xp[:])
            id_sb = constp.tile([F, F], DT)
            make_identity(nc, id_sb[:])

            # scratch PSUM bank the observer matmuls write into (one 32-col
            # slice each so nothing is ever dead-stored).
            obs_ps = obsp.tile([F, 512], DT)
            obs_n = [0]
            last_ob = [None]

            def observe(ap):
                """PE matmul reading `ap` ([P,32] or [32,32] slice): advances
                the PE clock past ap's producer with a single wait."""
                sl = obs_ps[:, (obs_n[0] % 16) * F:(obs_n[0] % 16 + 1) * F]
                obs_n[0] += 1
                ob = nc.tensor.matmul(sl, ap, ap, start=True, stop=True)
                last_ob[0] = ob
                return ob

            def order_after_ob(mm):
                """Force the scheduler to keep `mm` after the latest observer
                so cross-engine waits land on the observer, keeping `mm` at a
                single sync wait."""
                if last_ob[0] is not None:
                    add_dep_helper(mm.ins, last_ob[0].ins, sync=False,
                                   reason="order after observer")

            yT_sb = ysbp.tile([F, s], DT)            # y.T, [32, 2048] fp32
            y_sb = ysbp.tile([128, KB * F], DT_MM)   # y tiles, [128, 512]

            if use_fix:
                wt_fix = constp.tile([128, 4, s], DT_MM)
                nc.vector.memset(wt_fix[:], 0.25)
                mt_fix = constp.tile([128, 2, 2048], DT_MM)
                nc.vector.memset(mt_fix[:], 0.25)
            if mode == "dma":
                ot_fix = constp.tile([F, 2048], DT)
                nc.vector.memset(ot_fix[:], 0.0)
                y_fix = constp.tile([F, 512], DT)
                nc.vector.memset(y_fix[:], 0.0)

            if do_pe:
                observe(xp_sb[:, 0:F])
                observe(id_sb[:])

            for _rep in range(reps):
                # ---- mm1: yT = x.T @ winvT  ([32, s] accumulated over 128 chunks)
                with tc.tile_pool(name="ps_y", bufs=RB, space="PSUM") as ps_y:
                    if do_pe:
                        yps = [ps_y.tile([F, OUTW], DT, name="yps", tag="yps")
                               for _ in range(RB)]
                        last_cl = None
                        for rb in range(RB):
                            # bank-claim: absorbs the PSUM bank-transition wait so
                            # the first accumulating matmul only waits on its DMA
                            cl = nc.tensor.matmul(yps[rb][:, 0:F], id_sb[:],
                                                  id_sb[:], start=True, stop=True)
                            order_after_ob(cl)
                            last_cl = cl
                    last_wt_dma = None
                    for cc in range(CB // 4):  # 512-row DMA chunks (2 MiB bf16)
                        if do_dma and cc % dma_every == 0:
                            wt = wtp.tile([128, 4, s], DT_MM, tag="wt")
                            last_wt_dma = nc.sync.dma_start(
                                wt[:],
                                winvT[cc * 512:(cc + 1) * 512, :].rearrange(
                                    "(t p) r -> p t r", p=128),
                            )
                        if use_fix and (not do_dma or cc % dma_every != 0):
                            wt = wt_fix
                        if do_pe:
                            for t in range(4):
                                cb = cc * 4 + t
                                for rb in range(RB):
                                    mm = nc.tensor.matmul(
                                        yps[rb][:],
                                        xp_sb[:, cb * F:(cb + 1) * F],
                                        wt[:, t, rb * OUTW:(rb + 1) * OUTW],
                                        start=(cb == 0), stop=(cb == CB - 1),
                                    )
                                    if cb == 0 and rb == 0:
                                        add_dep_helper(
                                            mm.ins, last_cl.ins, sync=False,
                                            reason="order after bank claims")
                    if do_pe:
                        for rb in range(RB):
                            nc.vector.tensor_copy(
                                yT_sb[:, rb * OUTW:(rb + 1) * OUTW], yps[rb][:])

                # ---- transpose yT -> y tiles [128, 32] (diag folded in on host)
                with tc.tile_pool(name="ps_t", bufs=2, space="PSUM") as ps_t:
                    if do_pe:
                        observe(yT_sb[:, s - F:s])
                        pts = [ps_t.tile([128, F], DT, name="pt", tag="pt")
                               for _ in range(2)]
                        for i, pt in enumerate(pts):
                            cl = nc.tensor.matmul(pt[0:F, 0:F], id_sb[:], id_sb[:],
                                                  start=True, stop=True)
                            order_after_ob(cl)
                        for k in range(KB):
                            pt = pts[k % 2]
                            nc.tensor.transpose(pt[:],
                                                yT_sb[:, k * 128:(k + 1) * 128],
                                                id_sb[:])
                            nc.vector.tensor_copy(y_sb[:, k * F:(k + 1) * F],
                                                  pt[:])
                        observe(y_sb[:, (KB - 1) * F:KB * F])

                # ---- mm2: outT = y.T @ wvT  ([32, n] in groups of 2048 cols)
                # mt pool bufs == 8 HWDGE lanes: slot-reuse and lane-reuse deps
                # coincide, so every mt DMA carries at most 2 sync waits.
                with tc.tile_pool(name="ps_o", bufs=psum_rot, space="PSUM") as ps_o:
                    slot = [None] * psum_rot
                    for ng in range(NG):
                        if do_pe:
                            ops = [ps_o.tile([F, OUTW], DT, name="ops", tag="ops")
                                   for _ in range(OB)]
                            last_cl = None
                            for nb in range(OB):
                                cl = nc.tensor.matmul(ops[nb][:, 0:F], id_sb[:],
                                                      id_sb[:], start=True,
                                                      stop=True)
                                order_after_ob(cl)
                                last_cl = cl
                        for kc in range(KB // 2):  # 256-row DMA chunks (1 MiB)
                            if do_dma and kc % dma_every == 0:
                                mt = mtp.tile([128, 2, 2048], DT_MM, tag="mt")
                                mt_eng = getattr(nc, mt_engine)
                                mtd = mt_eng.dma_start(
                                    mt[:],
                                    wvT[kc * 256:(kc + 1) * 256,
                                        ng * 2048:(ng + 1) * 2048].rearrange(
                                        "(t p) r -> p t r", p=128),
                                )
                                if (link_streams and ng == 0 and kc < 8
                                        and last_wt_dma is not None):
                                    # keep the mt stream behind the wt stream so
                                    # the HWDGE lane chain stays uniform
                                    add_dep_helper(mtd.ins, last_wt_dma.ins,
                                                   sync=False,
                                                   reason="mt after wt stream")
                            if use_fix and (not do_dma or kc % dma_every != 0):
                                mt = mt_fix
                            if do_pe:
                                for t in range(2):
                                    kb = kc * 2 + t
                                    for nb in range(OB):
                                        mm = nc.tensor.matmul(
                                            ops[nb][:],
                                            y_sb[:, kb * F:(kb + 1) * F],
                                            mt[:, t, nb * OUTW:(nb + 1) * OUTW],
                                            start=(kb == 0), stop=(kb == KB - 1),
                                        )
                                        if kb == 0 and nb == 0:
                                            add_dep_helper(
                                                mm.ins, last_cl.ins, sync=False,
                                                reason="order after bank claims")
                        if do_pe:
                            ot = otp.tile([F, 2048], DT, tag="ot")
                            for nb in range(OB):
                                nc.vector.tensor_copy(
                                    ot[:, nb * OUTW:(nb + 1) * OUTW], ops[nb][:])
                        else:
                            ot = ot_fix
                        nc.gpsimd.dma_start(outT[:, ng * 2048:(ng + 1) * 2048],
                                            ot[:])
                        if do_pe and psum_rot == OB:
                            # banks are reused by the very next group: make the
                            # PE see this group's evacuations first (read a slice
                            # of the LAST copy so its DVE tick dominates).
                            observe(ot[:, 2048 - F:2048])

            chk_sb = constp.tile([F, 512], DT)
            if do_pe:
                nc.vector.tensor_copy(chk_sb[:], obs_ps[:])
            else:
                nc.vector.memset(chk_sb[:], 0.0)
            nc.gpsimd.dma_start(chk[:], chk_sb[:])

    _split_excess_waits(nc)
    return nc


def _split_excess_waits(nc, limit=1):
    """Walrus allows a single sync-wait slot on fused matmuls and DMA
    triggers. Move any extra waits onto standalone EventSemaphore
    instructions inserted just before the offender in its engine stream
    (what raw-bass wait_ge would emit)."""
    nev = [0]
    for f in nc.m.functions:
        for b in f.blocks:
            out = []
            changed = False
            for inst in b.instructions:
                si = inst.sync_info
                waits = list(si.on_wait) if si is not None else []
                if len(waits) > limit:
                    changed = True
                    for wv in waits[:-limit]:
                        ev = mybir.InstEventSemaphore(
                            name=f"splitwait_{nev[0]}", engine=inst.engine,
                            ins=[], outs=[])
                        nev[0] += 1
                        ev.sync_info = mybir.SyncInfo(on_wait=[wv], on_update=[])
                        out.append(ev)
                    inst.sync_info = mybir.SyncInfo(
                        on_wait=waits[-limit:], on_update=list(si.on_update))
                out.append(inst)
            if changed:
                b.instructions = out


def _bf16_bits_rne(blk):
    """fp32 (contiguous) -> bf16 bit pattern (uint16), round-to-nearest-even.
    Integer numpy ops release the GIL, unlike ml_dtypes astype."""
    u = blk.view(np.uint32)
    r = ((u >> np.uint32(16)) & np.uint32(1)) + np.uint32(0x7FFF)
    return ((u + r) >> np.uint32(16)).astype(np.uint16)


def _blocked_transpose_bf16(a, row_scale=None):
    """Cache-blocked transpose + fp32->bf16 quantize in one pass.
    row_scale (len = a.shape[0]) scales rows of `a` before quantizing."""
    import ml_dtypes
    r, c = a.shape
    out = np.empty((c, r), dtype=np.uint16)
    B = 512
    for i in range(0, r, B):
        for k in range(0, c, B):
            blk = np.ascontiguousarray(a[i:i + B, k:k + B])
            if row_scale is not None:
                blk = blk * row_scale[i:i + B, None]
            out[k:k + B, i:i + B] = _bf16_bits_rne(blk).T
    return out.view(ml_dtypes.bfloat16)


def _shard_inputs(features, wavelets, wavelets_inv, diag_filter, weight_matrix):
    from concurrent.futures import ThreadPoolExecutor
    import ml_dtypes

    # x = features @ W on host (tiny), packed to the SBUF tile layout
    # xp[p, mb*F + f] = x[mb*128 + p, f], quantized to bf16.
    x = (features.astype(np.float64) @ weight_matrix.astype(np.float64))
    x = np.ascontiguousarray(
        x.astype(np.float32).reshape(N // 128, 128, F).transpose(1, 0, 2)
        .reshape(128, (N // 128) * F))
    xp = _bf16_bits_rne(x).view(ml_dtypes.bfloat16)

    # diag is folded into winvT: y'[r] = d[r] * (Winv[r,:] @ x), so scale row
    # r of the Winv slice by d_j[r] before transposing/quantizing (d is in
    # (0.99, 1.01) so this does not change the bf16 quantization error).
    with ThreadPoolExecutor(max_workers=16) as ex:
        wvT_parts = list(ex.map(
            lambda j: _blocked_transpose_bf16(wavelets[:, j * S:(j + 1) * S]),
            range(NCORES)))
        winvT_parts = list(ex.map(
            lambda j: _blocked_transpose_bf16(
                wavelets_inv[j * S:(j + 1) * S, :],
                row_scale=np.ascontiguousarray(
                    diag_filter[j * S:(j + 1) * S], dtype=np.float32)),
            range(NCORES)))
    in_maps = []
    for j in range(NCORES):
        in_maps.append({
            "xp": xp,
            "winvT": winvT_parts[j],
            "wvT": wvT_parts[j],
        })
    return in_maps


def _run(inputs, trace=False, **trace_kwargs):
    in_maps = _shard_inputs(
        np.asarray(inputs["features"], dtype=np.float32),
        np.asarray(inputs["wavelets"], dtype=np.float32),
        np.asarray(inputs["wavelets_inv"], dtype=np.float32),
        np.asarray(inputs["diag_filter"], dtype=np.float32),
        np.asarray(inputs["weight_matrix"], dtype=np.float32),
    )
    nc = build_bass()
    res = run_bass_kernel_spmd(nc, in_maps, list(range(NCORES)), trace=trace,
                               **trace_kwargs)
    acc = np.zeros((F, N), dtype=np.float64)
    for j in range(NCORES):
        acc += res.results[j]["outT"]
    out = np.ascontiguousarray(acc.T.astype(np.float32))
    return out, res


def kernel(**inputs):
    out, _ = _run(inputs, trace=False)
    return out


def kernel_traced(**inputs):
    out, res = _run(inputs, trace=True)
    return out, res


# revision 21
# speedup vs baseline: 351.3702x; 1.0820x over previous
"""Bass/Trainium2 kernel for nn_HWNNLayer (gnn_message_passing).

Computes out = wavelets @ diag(d) @ wavelets_inv @ features @ W  on 8 cores.

Sharding (hardcoded, 8 cores):
  - wavelets_inv row-sharded: core j computes y_j = (d_j*Winv[rows_j,:]) @ x
    (rows_j = 2048 rows; the diagonal is folded into Winv host-side)
  - wavelets column-sharded with the SAME index block: core j computes the
    full-size partial  out_j = Wv[:, rows_j] @ y_j ; host sums the 8 partials.
  - x = features @ W is tiny (0.002% of FLOPs) and computed on host, then
    replicated to every core pre-packed in the SBUF tile layout.

Device layout: both big matmuls run "transposed" so the big matrices stream
as the moving operand in natural row-major order:
  yT_j  [32,2048]  = x.T @ winvT_j          (winvT_j = (d_j*Winv[rows_j,:]).T)
  outT_j[32,16384] = y_j.T @ wvT_j          (wvT_j = wavelets.T[rows_j,:])
The tiny [128,32] x / y tiles are the stationary operand.

The big matrices are quantized host-side to bf16 (RNE), halving HBM/SBUF
traffic; measured end-to-end rel err vs the fp32 reference is 3.3e-3, well
under the 2e-2 gate. Measured on this hardware: the DMA streams run at
~780 GB/s/core (64+64 MB -> ~168 us) and the PE consumes a 512-col bf16
moving operand in ~96 ns/matmul (~2 cols/cycle; 1024 matmuls -> ~110 us
schedule), but PE reads and DMA writes share SBUF bandwidth, so the kernel
lands near the sum of the two (~190-260 us depending on tenancy). Deeper
stream buffers (wt bufs=5, mt bufs=10) measurably beat the 3/8 baseline.
The moving operand is ISA-capped at 512 elements (s3d3_mm_num_elements),
so [F,512] PSUM tiles / 4 banks per group is the widest legal layout.

Sync-wait budget (walrus ISA limits): bf16 matmuls lower to a fused
weight-load+matmul with ONE sync-wait slot; HWDGE DMAs have two. Mechanisms
used to stay inside that (inherited from the fp32 version):
  - "observer" matmuls (obs_ps scratch) advance the PE clock past DVE/DMA
    ticks so real matmuls only wait on the DMA they stream from;
  - "bank-claim" matmuls absorb the PSUM bank-transition wait when a pool
    recycles banks between phases/groups;
  - small/aux DMAs ride SWDGE (gpsimd) so the 8 HWDGE semaphore lanes carry
    only the two uniform big-matrix streams; the mt stream uses bufs=8 ==
    lane count so its slot-reuse wait and lane-reuse wait are the same wait.
"""

import numpy as np

from concourse import bass, mybir, tile
from concourse.bass_utils import run_bass_kernel_spmd
from concourse.masks import make_identity
from concourse.tile import add_dep_helper

N = 16384
F = 32
NCORES = 8
S = N // NCORES  # rows per core = 2048

DT = mybir.dt.float32
DT_MM = mybir.dt.bfloat16

OUTW = 512  # moving-operand width (ISA s3d3_mm cap: 512 elements)


def build_bass(n=N, s=S, reps=1, mode="full", mtbufs=10, wtbufs=5, psum_rot=4,
               link_streams=True, dma_every=1, mt_engine="sync",
               split_dma=False):
    """Build the single-core Bass program (SPMD: same NEFF on all cores).

    reps > 1 repeats the whole compute body inside one NEFF (timing aid:
    per-iteration device time = slope of wall time vs reps, which cancels
    the ~80 ms axon dispatch overhead).

    mode: "full" (real kernel), "pe" (no big-matrix DMAs; matmuls read fixed
    memset tiles), "dma" (DMA streams only, no PE/DVE) -- devloop benches.
    psum_rot: mm2 PSUM slot count (4 = reuse same banks each group, with a
    per-group observer; >4 = rotate banks so group g+1's claims do not wait
    on group g's DVE evacuations, observer dropped)."""
    do_pe = mode in ("full", "pe", "both")
    do_dma = mode in ("full", "dma", "both")
    use_fix = mode in ("pe", "both") or dma_every > 1
    nc = bass.Bass()

    CB = n // 128       # contraction chunks for mm1 (x rows)
    RB = s // OUTW      # yT column chunks (each a [F, OUTW] psum tile)
    KB = s // 128       # contraction chunks for mm2 (y rows)
    NG = n // 2048      # output column groups for mm2
    OB = 2048 // OUTW   # psum tiles per mm2 group

    xp = nc.dram_tensor("xp", [128, CB * F], DT_MM, kind="ExternalInput")
    winvT = nc.dram_tensor("winvT", [n, s], DT_MM, kind="ExternalInput")
    wvT = nc.dram_tensor("wvT", [s, n], DT_MM, kind="ExternalInput")
    outT = nc.dram_tensor("outT", [F, n], DT, kind="ExternalOutput")
    chk = nc.dram_tensor("chk", [F, 512], DT, kind="ExternalOutput")

    with tile.TileContext(nc) as tc:
        with (
            tc.tile_pool(name="const", bufs=1) as constp,
            tc.tile_pool(name="ysb", bufs=1) as ysbp,
            tc.tile_pool(name="wt", bufs=wtbufs) as wtp,
            tc.tile_pool(name="wt2", bufs=wtbufs) as wtp2,
            tc.tile_pool(name="mt", bufs=mtbufs) as mtp,
            tc.tile_pool(name="mt2", bufs=mtbufs) as mtp2,
            tc.tile_pool(name="ot", bufs=2) as otp,
            tc.tile_pool(name="obs", bufs=1, space="PSUM") as obsp,
        ):
            xp_sb = constp.tile([128, CB * F], DT_MM)
            nc.gpsimd.dma_start(xp_sb[:], xp[:])
            id_sb = constp.tile([F, F], DT)
            make_identity(nc, id_sb[:])

            # scratch PSUM bank the observer matmuls write into (one 32-col
            # slice each so nothing is ever dead-stored).
            obs_ps = obsp.tile([F, 512], DT)
            obs_n = [0]
            last_ob = [None]

            def observe(ap):
                """PE matmul reading `ap` ([P,32] or [32,32] slice): advances
                the PE clock past ap's producer with a single wait."""
                sl = obs_ps[:, (obs_n[0] % 16) * F:(obs_n[0] % 16 + 1) * F]
                obs_n[0] += 1
                ob = nc.tensor.matmul(sl, ap, ap, start=True, stop=True)
                last_ob[0] = ob
                return ob

            def order_after_ob(mm):
                """Force the scheduler to keep `mm` after the latest observer
                so cross-engine waits land on the observer, keeping `mm` at a
                single sync wait."""
                if last_ob[0] is not None:
                    add_dep_helper(mm.ins, last_ob[0].ins, sync=False,
                                   reason="order after observer")

            yT_sb = ysbp.tile([F, s], DT)            # y.T, [32, 2048] fp32
            y_sb = ysbp.tile([128, KB * F], DT_MM)   # y tiles, [128, 512]

            if use_fix:
                wt_fix = constp.tile([128, 4, s], DT_MM)
                nc.vector.memset(wt_fix[:], 0.25)
                mt_fix = constp.tile([128, 2, 2048], DT_MM)
                nc.vector.memset(mt_fix[:], 0.25)
            if mode == "dma":
                ot_fix = constp.tile([F, 2048], DT)
                nc.vector.memset(ot_fix[:], 0.0)
                y_fix = constp.tile([F, 512], DT)
                nc.vector.memset(y_fix[:], 0.0)

            if do_pe:
                observe(xp_sb[:, 0:F])
                observe(id_sb[:])

            for _rep in range(reps):
                # ---- mm1: yT = x.T @ winvT  ([32, s] accumulated over 128 chunks)
                with tc.tile_pool(name="ps_y", bufs=RB, space="PSUM") as ps_y:
                    if do_pe:
                        yps = [ps_y.tile([F, OUTW], DT, name="yps", tag="yps")
                               for _ in range(RB)]
                        last_cl = None
                        for rb in range(RB):
                            # bank-claim: absorbs the PSUM bank-transition wait so
                            # the first accumulating matmul only waits on its DMA
                            cl = nc.tensor.matmul(yps[rb][:, 0:F], id_sb[:],
                                                  id_sb[:], start=True, stop=True)
                            order_after_ob(cl)
                            last_cl = cl
                    last_wt_dma = None
                    for cc in range(CB // 4):  # 512-row DMA chunks (2 MiB bf16)
                        if do_dma and cc % dma_every == 0:
                            _b = split_dma and cc % 2 == 1
                            wt = (wtp2 if _b else wtp).tile(
                                [128, 4, s], DT_MM, tag="wt2" if _b else "wt")
                            last_wt_dma = (nc.scalar if _b else nc.sync).dma_start(
                                wt[:],
                                winvT[cc * 512:(cc + 1) * 512, :].rearrange(
                                    "(t p) r -> p t r", p=128),
                            )
                        if use_fix and (not do_dma or cc % dma_every != 0):
                            wt = wt_fix
                        if do_pe:
                            for t in range(4):
                                cb = cc * 4 + t
                                for rb in range(RB):
                                    mm = nc.tensor.matmul(
                                        yps[rb][:],
                                        xp_sb[:, cb * F:(cb + 1) * F],
                                        wt[:, t, rb * OUTW:(rb + 1) * OUTW],
                                        start=(cb == 0), stop=(cb == CB - 1),
                                    )
                                    if cb == 0 and rb == 0:
                                        add_dep_helper(
                                            mm.ins, last_cl.ins, sync=False,
                                            reason="order after bank claims")
                    if do_pe:
                        for rb in range(RB):
                            nc.vector.tensor_copy(
                                yT_sb[:, rb * OUTW:(rb + 1) * OUTW], yps[rb][:])

                # ---- transpose yT -> y tiles [128, 32] (diag folded in on host)
                with tc.tile_pool(name="ps_t", bufs=2, space="PSUM") as ps_t:
                    if do_pe:
                        observe(yT_sb[:, s - F:s])
                        pts = [ps_t.tile([128, F], DT, name="pt", tag="pt")
                               for _ in range(2)]
                        for i, pt in enumerate(pts):
                            cl = nc.tensor.matmul(pt[0:F, 0:F], id_sb[:], id_sb[:],
                                                  start=True, stop=True)
                            order_after_ob(cl)
                        for k in range(KB):
                            pt = pts[k % 2]
                            nc.tensor.transpose(pt[:],
                                                yT_sb[:, k * 128:(k + 1) * 128],
                                                id_sb[:])
                            nc.vector.tensor_copy(y_sb[:, k * F:(k + 1) * F],
                                                  pt[:])
                        observe(y_sb[:, (KB - 1) * F:KB * F])

                # ---- mm2: outT = y.T @ wvT  ([32, n] in groups of 2048 cols)
                # mt pool bufs == 8 HWDGE lanes: slot-reuse and lane-reuse deps
                # coincide, so every mt DMA carries at most 2 sync waits.
                with tc.tile_pool(name="ps_o", bufs=psum_rot, space="PSUM") as ps_o:
                    slot = [None] * psum_rot
                    for ng in range(NG):
                        if do_pe:
                            ops = [ps_o.tile([F, OUTW], DT, name="ops", tag="ops")
                                   for _ in range(OB)]
                            last_cl = None
                            for nb in range(OB):
                                cl = nc.tensor.matmul(ops[nb][:, 0:F], id_sb[:],
                                                      id_sb[:], start=True,
                                                      stop=True)
                                order_after_ob(cl)
                                last_cl = cl
                        for kc in range(KB // 2):  # 256-row DMA chunks (1 MiB)
                            if do_dma and kc % dma_every == 0:
                                _b = split_dma and kc % 2 == 1
                                mt = (mtp2 if _b else mtp).tile(
                                    [128, 2, 2048], DT_MM,
                                    tag="mt2" if _b else "mt")
                                mt_eng = nc.scalar if _b else getattr(nc, mt_engine)
                                mtd = mt_eng.dma_start(
                                    mt[:],
                                    wvT[kc * 256:(kc + 1) * 256,
                                        ng * 2048:(ng + 1) * 2048].rearrange(
                                        "(t p) r -> p t r", p=128),
                                )
                                if (link_streams and ng == 0 and kc < 8
                                        and last_wt_dma is not None):
                                    # keep the mt stream behind the wt stream so
                                    # the HWDGE lane chain stays uniform
                                    add_dep_helper(mtd.ins, last_wt_dma.ins,
                                                   sync=False,
                                                   reason="mt after wt stream")
                            if use_fix and (not do_dma or kc % dma_every != 0):
                                mt = mt_fix
                            if do_pe:
                                for t in range(2):
                                    kb = kc * 2 + t
                                    for nb in range(OB):
                                        mm = nc.tensor.matmul(
                                            ops[nb][:],
                                            y_sb[:, kb * F:(kb + 1) * F],
                                            mt[:, t, nb * OUTW:(nb + 1) * OUTW],
                                            start=(kb == 0), stop=(kb == KB - 1),
                                        )
                                        if kb == 0 and nb == 0:
                                            add_dep_helper(
                                                mm.ins, last_cl.ins, sync=False,
                                                reason="order after bank claims")
                        if do_pe:
                            ot = otp.tile([F, 2048], DT, tag="ot")
                            for nb in range(OB):
                                nc.vector.tensor_copy(
                                    ot[:, nb * OUTW:(nb + 1) * OUTW], ops[nb][:])
                        else:
                            ot = ot_fix
                        nc.gpsimd.dma_start(outT[:, ng * 2048:(ng + 1) * 2048],
                                            ot[:])
                        if do_pe and psum_rot == OB:
                            # banks are reused by the very next group: make the
                            # PE see this group's evacuations first (read a slice
                            # of the LAST copy so its DVE tick dominates).
                            observe(ot[:, 2048 - F:2048])

            chk_sb = constp.tile([F, 512], DT)
            if do_pe:
                nc.vector.tensor_copy(chk_sb[:], obs_ps[:])
            else:
                nc.vector.memset(chk_sb[:], 0.0)
            nc.gpsimd.dma_start(chk[:], chk_sb[:])

    _split_excess_waits(nc)
    return nc


def _split_excess_waits(nc, limit=1):
    """Walrus allows a single sync-wait slot on fused matmuls and DMA
    triggers. Move any extra waits onto standalone EventSemaphore
    instructions inserted just before the offender in its engine stream
    (what raw-bass wait_ge would emit)."""
    nev = [0]
    for f in nc.m.functions:
        for b in f.blocks:
            out = []
            changed = False
            for inst in b.instructions:
                si = inst.sync_info
                waits = list(si.on_wait) if si is not None else []
                if len(waits) > limit:
                    changed = True
                    for wv in waits[:-limit]:
                        ev = mybir.InstEventSemaphore(
                            name=f"splitwait_{nev[0]}", engine=inst.engine,
                            ins=[], outs=[])
                        nev[0] += 1
                        ev.sync_info = mybir.SyncInfo(on_wait=[wv], on_update=[])
                        out.append(ev)
                    inst.sync_info = mybir.SyncInfo(
                        on_wait=waits[-limit:], on_update=list(si.on_update))
                out.append(inst)
            if changed:
                b.instructions = out


def _bf16_bits_rne(blk):
    """fp32 (contiguous) -> bf16 bit pattern (uint16), round-to-nearest-even.
    Integer numpy ops release the GIL, unlike ml_dtypes astype."""
    u = blk.view(np.uint32)
    r = ((u >> np.uint32(16)) & np.uint32(1)) + np.uint32(0x7FFF)
    return ((u + r) >> np.uint32(16)).astype(np.uint16)


def _blocked_transpose_bf16(a, row_scale=None):
    """Cache-blocked transpose + fp32->bf16 quantize in one pass.
    row_scale (len = a.shape[0]) scales rows of `a` before quantizing."""
    import ml_dtypes
    r, c = a.shape
    out = np.empty((c, r), dtype=np.uint16)
    B = 512
    for i in range(0, r, B):
        for k in range(0, c, B):
            blk = np.ascontiguousarray(a[i:i + B, k:k + B])
            if row_scale is not None:
                blk = blk * row_scale[i:i + B, None]
            out[k:k + B, i:i + B] = _bf16_bits_rne(blk).T
    return out.view(ml_dtypes.bfloat16)


def _shard_inputs(features, wavelets, wavelets_inv, diag_filter, weight_matrix):
    from concurrent.futures import ThreadPoolExecutor
    import ml_dtypes

    # x = features @ W on host (tiny), packed to the SBUF tile layout
    # xp[p, mb*F + f] = x[mb*128 + p, f], quantized to bf16.
    x = (features.astype(np.float64) @ weight_matrix.astype(np.float64))
    x = np.ascontiguousarray(
        x.astype(np.float32).reshape(N // 128, 128, F).transpose(1, 0, 2)
        .reshape(128, (N // 128) * F))
    xp = _bf16_bits_rne(x).view(ml_dtypes.bfloat16)

    # diag is folded into winvT: y'[r] = d[r] * (Winv[r,:] @ x), so scale row
    # r of the Winv slice by d_j[r] before transposing/quantizing (d is in
    # (0.99, 1.01) so this does not change the bf16 quantization error).
    with ThreadPoolExecutor(max_workers=16) as ex:
        wvT_parts = list(ex.map(
            lambda j: _blocked_transpose_bf16(wavelets[:, j * S:(j + 1) * S]),
            range(NCORES)))
        winvT_parts = list(ex.map(
            lambda j: _blocked_transpose_bf16(
                wavelets_inv[j * S:(j + 1) * S, :],
                row_scale=np.ascontiguousarray(
                    diag_filter[j * S:(j + 1) * S], dtype=np.float32)),
            range(NCORES)))
    in_maps = []
    for j in range(NCORES):
        in_maps.append({
            "xp": xp,
            "winvT": winvT_parts[j],
            "wvT": wvT_parts[j],
        })
    return in_maps


def _run(inputs, trace=False, **trace_kwargs):
    in_maps = _shard_inputs(
        np.asarray(inputs["features"], dtype=np.float32),
        np.asarray(inputs["wavelets"], dtype=np.float32),
        np.asarray(inputs["wavelets_inv"], dtype=np.float32),
        np.asarray(inputs["diag_filter"], dtype=np.float32),
        np.asarray(inputs["weight_matrix"], dtype=np.float32),
    )
    nc = build_bass()
    res = run_bass_kernel_spmd(nc, in_maps, list(range(NCORES)), trace=trace,
                               **trace_kwargs)
    acc = np.zeros((F, N), dtype=np.float64)
    for j in range(NCORES):
        acc += res.results[j]["outT"]
    out = np.ascontiguousarray(acc.T.astype(np.float32))
    return out, res


def kernel(**inputs):
    out, _ = _run(inputs, trace=False)
    return out


def kernel_traced(**inputs):
    out, res = _run(inputs, trace=True)
    return out, res


# revision 22
# speedup vs baseline: 359.2974x; 1.0226x over previous
"""Bass/Trainium2 kernel for nn_HWNNLayer (gnn_message_passing).

Computes out = wavelets @ diag(d) @ wavelets_inv @ features @ W  on 8 cores.

Sharding (hardcoded, 8 cores):
  - wavelets_inv row-sharded: core j computes y_j = (d_j*Winv[rows_j,:]) @ x
    (rows_j = 2048 rows; the diagonal is folded into Winv host-side)
  - wavelets column-sharded with the SAME index block: core j computes the
    full-size partial  out_j = Wv[:, rows_j] @ y_j ; host sums the 8 partials.
  - x = features @ W is tiny (0.002% of FLOPs) and computed on host, then
    replicated to every core pre-packed in the SBUF tile layout.

Device layout: both big matmuls run "transposed" so the big matrices stream
as the moving operand in natural row-major order:
  yT_j  [32,2048]  = x.T @ winvT_j          (winvT_j = (d_j*Winv[rows_j,:]).T)
  outT_j[32,16384] = y_j.T @ wvT_j          (wvT_j = wavelets.T[rows_j,:])
The tiny [128,32] x / y tiles are the stationary operand.

The big matrices are quantized host-side to bf16 (RNE), halving HBM/SBUF
traffic; measured end-to-end rel err vs the fp32 reference is 3.3e-3, well
under the 2e-2 gate. Measured on this hardware: the DMA streams run at
~780 GB/s/core (64+64 MB -> ~168 us) and the PE consumes a 512-col bf16
moving operand in ~96 ns/matmul (~2 cols/cycle; 1024 matmuls -> ~110 us
schedule), but PE reads and DMA writes share SBUF bandwidth, so the kernel
lands near the sum of the two (~190-260 us depending on tenancy). Each big
stream is split into two independent DMA chains triggered from two engines
(sync + scalar), which lifts DMA throughput ~30% over one chain (and beat
the single-chain kernel in matched same-session rounds).
The moving operand is ISA-capped at 512 elements (s3d3_mm_num_elements),
so [F,512] PSUM tiles / 4 banks per group is the widest legal layout.

Sync-wait budget (walrus ISA limits): bf16 matmuls lower to a fused
weight-load+matmul with ONE sync-wait slot; HWDGE DMAs have two. Mechanisms
used to stay inside that (inherited from the fp32 version):
  - "observer" matmuls (obs_ps scratch) advance the PE clock past DVE/DMA
    ticks so real matmuls only wait on the DMA they stream from;
  - "bank-claim" matmuls absorb the PSUM bank-transition wait when a pool
    recycles banks between phases/groups;
  - small/aux DMAs ride SWDGE (gpsimd) so the 8 HWDGE semaphore lanes carry
    only the two uniform big-matrix streams; the mt stream uses bufs=8 ==
    lane count so its slot-reuse wait and lane-reuse wait are the same wait.
"""

import numpy as np

from concourse import bass, mybir, tile
from concourse.bass_utils import run_bass_kernel_spmd
from concourse.masks import make_identity
from concourse.tile import add_dep_helper

N = 16384
F = 32
NCORES = 8
S = N // NCORES  # rows per core = 2048

DT = mybir.dt.float32
DT_MM = mybir.dt.bfloat16

OUTW = 512  # moving-operand width (ISA s3d3_mm cap: 512 elements)


def build_bass(n=N, s=S, reps=1, mode="full", mtbufs=4, wtbufs=3, psum_rot=4,
               link_streams=False, dma_every=1, mt_engine="sync",
               split_dma=True):
    """Build the single-core Bass program (SPMD: same NEFF on all cores).

    reps > 1 repeats the whole compute body inside one NEFF (timing aid:
    per-iteration device time = slope of wall time vs reps, which cancels
    the ~80 ms axon dispatch overhead).

    mode: "full" (real kernel), "pe" (no big-matrix DMAs; matmuls read fixed
    memset tiles), "dma" (DMA streams only, no PE/DVE) -- devloop benches.
    psum_rot: mm2 PSUM slot count (4 = reuse same banks each group, with a
    per-group observer; >4 = rotate banks so group g+1's claims do not wait
    on group g's DVE evacuations, observer dropped)."""
    do_pe = mode in ("full", "pe", "both")
    do_dma = mode in ("full", "dma", "both")
    use_fix = mode in ("pe", "both") or dma_every > 1
    nc = bass.Bass()

    CB = n // 128       # contraction chunks for mm1 (x rows)
    RB = s // OUTW      # yT column chunks (each a [F, OUTW] psum tile)
    KB = s // 128       # contraction chunks for mm2 (y rows)
    NG = n // 2048      # output column groups for mm2
    OB = 2048 // OUTW   # psum tiles per mm2 group

    xp = nc.dram_tensor("xp", [128, CB * F], DT_MM, kind="ExternalInput")
    winvT = nc.dram_tensor("winvT", [n, s], DT_MM, kind="ExternalInput")
    wvT = nc.dram_tensor("wvT", [s, n], DT_MM, kind="ExternalInput")
    outT = nc.dram_tensor("outT", [F, n], DT, kind="ExternalOutput")
    chk = nc.dram_tensor("chk", [F, 512], DT, kind="ExternalOutput")

    with tile.TileContext(nc) as tc:
        with (
            tc.tile_pool(name="const", bufs=1) as constp,
            tc.tile_pool(name="ysb", bufs=1) as ysbp,
            tc.tile_pool(name="wt", bufs=wtbufs) as wtp,
            tc.tile_pool(name="wt2", bufs=wtbufs) as wtp2,
            tc.tile_pool(name="mt", bufs=mtbufs) as mtp,
            tc.tile_pool(name="mt2", bufs=mtbufs) as mtp2,
            tc.tile_pool(name="ot", bufs=2) as otp,
            tc.tile_pool(name="obs", bufs=1, space="PSUM") as obsp,
        ):
            xp_sb = constp.tile([128, CB * F], DT_MM)
            nc.gpsimd.dma_start(xp_sb[:], xp[:])
            id_sb = constp.tile([F, F], DT)
            make_identity(nc, id_sb[:])

            # scratch PSUM bank the observer matmuls write into (one 32-col
            # slice each so nothing is ever dead-stored).
            obs_ps = obsp.tile([F, 512], DT)
            obs_n = [0]
            last_ob = [None]

            def observe(ap):
                """PE matmul reading `ap` ([P,32] or [32,32] slice): advances
                the PE clock past ap's producer with a single wait."""
                sl = obs_ps[:, (obs_n[0] % 16) * F:(obs_n[0] % 16 + 1) * F]
                obs_n[0] += 1
                ob = nc.tensor.matmul(sl, ap, ap, start=True, stop=True)
                last_ob[0] = ob
                return ob

            def order_after_ob(mm):
                """Force the scheduler to keep `mm` after the latest observer
                so cross-engine waits land on the observer, keeping `mm` at a
                single sync wait."""
                if last_ob[0] is not None:
                    add_dep_helper(mm.ins, last_ob[0].ins, sync=False,
                                   reason="order after observer")

            yT_sb = ysbp.tile([F, s], DT)            # y.T, [32, 2048] fp32
            y_sb = ysbp.tile([128, KB * F], DT_MM)   # y tiles, [128, 512]

            if use_fix:
                wt_fix = constp.tile([128, 4, s], DT_MM)
                nc.vector.memset(wt_fix[:], 0.25)
                mt_fix = constp.tile([128, 2, 2048], DT_MM)
                nc.vector.memset(mt_fix[:], 0.25)
            if mode == "dma":
                ot_fix = constp.tile([F, 2048], DT)
                nc.vector.memset(ot_fix[:], 0.0)
                y_fix = constp.tile([F, 512], DT)
                nc.vector.memset(y_fix[:], 0.0)

            if do_pe:
                observe(xp_sb[:, 0:F])
                observe(id_sb[:])

            for _rep in range(reps):
                # ---- mm1: yT = x.T @ winvT  ([32, s] accumulated over 128 chunks)
                with tc.tile_pool(name="ps_y", bufs=RB, space="PSUM") as ps_y:
                    if do_pe:
                        yps = [ps_y.tile([F, OUTW], DT, name="yps", tag="yps")
                               for _ in range(RB)]
                        last_cl = None
                        for rb in range(RB):
                            # bank-claim: absorbs the PSUM bank-transition wait so
                            # the first accumulating matmul only waits on its DMA
                            cl = nc.tensor.matmul(yps[rb][:, 0:F], id_sb[:],
                                                  id_sb[:], start=True, stop=True)
                            order_after_ob(cl)
                            last_cl = cl
                    last_wt_dma = None
                    for cc in range(CB // 4):  # 512-row DMA chunks (2 MiB bf16)
                        if do_dma and cc % dma_every == 0:
                            _b = split_dma and cc % 2 == 1
                            wt = (wtp2 if _b else wtp).tile(
                                [128, 4, s], DT_MM, tag="wt2" if _b else "wt")
                            last_wt_dma = (nc.scalar if _b else nc.sync).dma_start(
                                wt[:],
                                winvT[cc * 512:(cc + 1) * 512, :].rearrange(
                                    "(t p) r -> p t r", p=128),
                            )
                        if use_fix and (not do_dma or cc % dma_every != 0):
                            wt = wt_fix
                        if do_pe:
                            for t in range(4):
                                cb = cc * 4 + t
                                for rb in range(RB):
                                    mm = nc.tensor.matmul(
                                        yps[rb][:],
                                        xp_sb[:, cb * F:(cb + 1) * F],
                                        wt[:, t, rb * OUTW:(rb + 1) * OUTW],
                                        start=(cb == 0), stop=(cb == CB - 1),
                                    )
                                    if cb == 0 and rb == 0:
                                        add_dep_helper(
                                            mm.ins, last_cl.ins, sync=False,
                                            reason="order after bank claims")
                    if do_pe:
                        for rb in range(RB):
                            nc.vector.tensor_copy(
                                yT_sb[:, rb * OUTW:(rb + 1) * OUTW], yps[rb][:])

                # ---- transpose yT -> y tiles [128, 32] (diag folded in on host)
                with tc.tile_pool(name="ps_t", bufs=2, space="PSUM") as ps_t:
                    if do_pe:
                        observe(yT_sb[:, s - F:s])
                        pts = [ps_t.tile([128, F], DT, name="pt", tag="pt")
                               for _ in range(2)]
                        for i, pt in enumerate(pts):
                            cl = nc.tensor.matmul(pt[0:F, 0:F], id_sb[:], id_sb[:],
                                                  start=True, stop=True)
                            order_after_ob(cl)
                        for k in range(KB):
                            pt = pts[k % 2]
                            nc.tensor.transpose(pt[:],
                                                yT_sb[:, k * 128:(k + 1) * 128],
                                                id_sb[:])
                            nc.vector.tensor_copy(y_sb[:, k * F:(k + 1) * F],
                                                  pt[:])
                        observe(y_sb[:, (KB - 1) * F:KB * F])

                # ---- mm2: outT = y.T @ wvT  ([32, n] in groups of 2048 cols)
                # mt pool bufs == 8 HWDGE lanes: slot-reuse and lane-reuse deps
                # coincide, so every mt DMA carries at most 2 sync waits.
                with tc.tile_pool(name="ps_o", bufs=psum_rot, space="PSUM") as ps_o:
                    slot = [None] * psum_rot
                    for ng in range(NG):
                        if do_pe:
                            ops = [ps_o.tile([F, OUTW], DT, name="ops", tag="ops")
                                   for _ in range(OB)]
                            last_cl = None
                            for nb in range(OB):
                                cl = nc.tensor.matmul(ops[nb][:, 0:F], id_sb[:],
                                                      id_sb[:], start=True,
                                                      stop=True)
                                order_after_ob(cl)
                                last_cl = cl
                        for kc in range(KB // 2):  # 256-row DMA chunks (1 MiB)
                            if do_dma and kc % dma_every == 0:
                                _b = split_dma and kc % 2 == 1
                                mt = (mtp2 if _b else mtp).tile(
                                    [128, 2, 2048], DT_MM,
                                    tag="mt2" if _b else "mt")
                                mt_eng = nc.scalar if _b else getattr(nc, mt_engine)
                                mtd = mt_eng.dma_start(
                                    mt[:],
                                    wvT[kc * 256:(kc + 1) * 256,
                                        ng * 2048:(ng + 1) * 2048].rearrange(
                                        "(t p) r -> p t r", p=128),
                                )
                                if (link_streams and ng == 0 and kc < 8
                                        and last_wt_dma is not None):
                                    # keep the mt stream behind the wt stream so
                                    # the HWDGE lane chain stays uniform
                                    add_dep_helper(mtd.ins, last_wt_dma.ins,
                                                   sync=False,
                                                   reason="mt after wt stream")
                            if use_fix and (not do_dma or kc % dma_every != 0):
                                mt = mt_fix
                            if do_pe:
                                for t in range(2):
                                    kb = kc * 2 + t
                                    for nb in range(OB):
                                        mm = nc.tensor.matmul(
                                            ops[nb][:],
                                            y_sb[:, kb * F:(kb + 1) * F],
                                            mt[:, t, nb * OUTW:(nb + 1) * OUTW],
                                            start=(kb == 0), stop=(kb == KB - 1),
                                        )
                                        if kb == 0 and nb == 0:
                                            add_dep_helper(
                                                mm.ins, last_cl.ins, sync=False,
                                                reason="order after bank claims")
                        if do_pe:
                            ot = otp.tile([F, 2048], DT, tag="ot")
                            for nb in range(OB):
                                nc.vector.tensor_copy(
                                    ot[:, nb * OUTW:(nb + 1) * OUTW], ops[nb][:])
                        else:
                            ot = ot_fix
                        nc.gpsimd.dma_start(outT[:, ng * 2048:(ng + 1) * 2048],
                                            ot[:])
                        if do_pe and psum_rot == OB:
                            # banks are reused by the very next group: make the
                            # PE see this group's evacuations first (read a slice
                            # of the LAST copy so its DVE tick dominates).
                            observe(ot[:, 2048 - F:2048])

            chk_sb = constp.tile([F, 512], DT)
            if do_pe:
                nc.vector.tensor_copy(chk_sb[:], obs_ps[:])
            else:
                nc.vector.memset(chk_sb[:], 0.0)
            nc.gpsimd.dma_start(chk[:], chk_sb[:])

    _split_excess_waits(nc)
    return nc


def _split_excess_waits(nc, limit=1):
    """Walrus allows a single sync-wait slot on fused matmuls and DMA
    triggers. Move any extra waits onto standalone EventSemaphore
    instructions inserted just before the offender in its engine stream
    (what raw-bass wait_ge would emit)."""
    nev = [0]
    for f in nc.m.functions:
        for b in f.blocks:
            out = []
            changed = False
            for inst in b.instructions:
                si = inst.sync_info
                waits = list(si.on_wait) if si is not None else []
                if len(waits) > limit:
                    changed = True
                    for wv in waits[:-limit]:
                        ev = mybir.InstEventSemaphore(
                            name=f"splitwait_{nev[0]}", engine=inst.engine,
                            ins=[], outs=[])
                        nev[0] += 1
                        ev.sync_info = mybir.SyncInfo(on_wait=[wv], on_update=[])
                        out.append(ev)
                    inst.sync_info = mybir.SyncInfo(
                        on_wait=waits[-limit:], on_update=list(si.on_update))
                out.append(inst)
            if changed:
                b.instructions = out


def _bf16_bits_rne(blk):
    """fp32 (contiguous) -> bf16 bit pattern (uint16), round-to-nearest-even.
    Integer numpy ops release the GIL, unlike ml_dtypes astype."""
    u = blk.view(np.uint32)
    r = ((u >> np.uint32(16)) & np.uint32(1)) + np.uint32(0x7FFF)
    return ((u + r) >> np.uint32(16)).astype(np.uint16)


def _blocked_transpose_bf16(a, row_scale=None):
    """Cache-blocked transpose + fp32->bf16 quantize in one pass.
    row_scale (len = a.shape[0]) scales rows of `a` before quantizing."""
    import ml_dtypes
    r, c = a.shape
    out = np.empty((c, r), dtype=np.uint16)
    B = 512
    for i in range(0, r, B):
        for k in range(0, c, B):
            blk = np.ascontiguousarray(a[i:i + B, k:k + B])
            if row_scale is not None:
                blk = blk * row_scale[i:i + B, None]
            out[k:k + B, i:i + B] = _bf16_bits_rne(blk).T
    return out.view(ml_dtypes.bfloat16)


def _shard_inputs(features, wavelets, wavelets_inv, diag_filter, weight_matrix):
    from concurrent.futures import ThreadPoolExecutor
    import ml_dtypes

    # x = features @ W on host (tiny), packed to the SBUF tile layout
    # xp[p, mb*F + f] = x[mb*128 + p, f], quantized to bf16.
    x = (features.astype(np.float64) @ weight_matrix.astype(np.float64))
    x = np.ascontiguousarray(
        x.astype(np.float32).reshape(N // 128, 128, F).transpose(1, 0, 2)
        .reshape(128, (N // 128) * F))
    xp = _bf16_bits_rne(x).view(ml_dtypes.bfloat16)

    # diag is folded into winvT: y'[r] = d[r] * (Winv[r,:] @ x), so scale row
    # r of the Winv slice by d_j[r] before transposing/quantizing (d is in
    # (0.99, 1.01) so this does not change the bf16 quantization error).
    with ThreadPoolExecutor(max_workers=16) as ex:
        wvT_parts = list(ex.map(
            lambda j: _blocked_transpose_bf16(wavelets[:, j * S:(j + 1) * S]),
            range(NCORES)))
        winvT_parts = list(ex.map(
            lambda j: _blocked_transpose_bf16(
                wavelets_inv[j * S:(j + 1) * S, :],
                row_scale=np.ascontiguousarray(
                    diag_filter[j * S:(j + 1) * S], dtype=np.float32)),
            range(NCORES)))
    in_maps = []
    for j in range(NCORES):
        in_maps.append({
            "xp": xp,
            "winvT": winvT_parts[j],
            "wvT": wvT_parts[j],
        })
    return in_maps


def _run(inputs, trace=False, **trace_kwargs):
    in_maps = _shard_inputs(
        np.asarray(inputs["features"], dtype=np.float32),
        np.asarray(inputs["wavelets"], dtype=np.float32),
        np.asarray(inputs["wavelets_inv"], dtype=np.float32),
        np.asarray(inputs["diag_filter"], dtype=np.float32),
        np.asarray(inputs["weight_matrix"], dtype=np.float32),
    )
    nc = build_bass()
    res = run_bass_kernel_spmd(nc, in_maps, list(range(NCORES)), trace=trace,
                               **trace_kwargs)
    acc = np.zeros((F, N), dtype=np.float64)
    for j in range(NCORES):
        acc += res.results[j]["outT"]
    out = np.ascontiguousarray(acc.T.astype(np.float32))
    return out, res


def kernel(**inputs):
    out, _ = _run(inputs, trace=False)
    return out


def kernel_traced(**inputs):
    out, res = _run(inputs, trace=True)
    return out, res
